# revision 17
# baseline (speedup 1.0000x reference)
"""GNN message-passing kernel for Trainium2 (8 NeuronCores, SPMD) — v3.

Computation (see reference):
  h1 = tanh(A x @ W1 + b1)          [A(xW) = (Ax)W]
  s2 = h1 @ W2
  h2 = tanh(A s2 + b2)
  ht = logmap0(proj(h2))            (rowwise scale)
  pooled = segment mean over seg_ids, then expmap0/proj (host epilogue)

Sharding: nodes split contiguously over cores (dst-shard), SHARD=16384.

v3 structure per core:
  L1  streams host-pregathered x rows (bf16) in per-block windows (one
      128-edge window per dst block, pads masked).  One DVE is_equal
      one-hot per window feeds one PE matmul into the node-major block
      acc [slot, 256].  Epilogue transposes the acc, h1T = W1^T accT
      (Act tanh with per-partition bias), s2T = (4*W2)^T h1T, transpose
      to node-major fp8e3 s2 rows (scaled x4 for fp8 range).
  Exchange: FOUR fp8 AllGathers (one per quarter of the node space,
      issued as soon as that quarter's L1 blocks finish), each expanded
      by a DRAM->DRAM DMA into a 256B-strided padded table so dma_gather
      can fetch 256-byte descriptors legally.
  L2  four chunks (= exchange stages = int16 sub-tables).  Contiguous
      per-table gather streams (8x128 rows per call, batched idx loads),
      DVE slot masks, feature-major accumulation (lhsT = gathered fp8
      rows), PSUM acc per (group,block) within a chunk, SBUF partial
      across chunks; finalize = add + Act tanh(scale=0.25, bias) ->
      norms via PE ones-matmul -> batched logmap scale -> transpose ->
      pooling via fp16 seg masks.
"""

import numpy as np
import ml_dtypes
from contextlib import ExitStack

import concourse.bass as bass
import concourse.tile as tile
import concourse.bacc as bacc
from concourse import mybir

BF16 = mybir.dt.bfloat16
FP16 = mybir.dt.float16
FP8 = mybir.dt.float8e3
F32 = mybir.dt.float32
I16 = mybir.dt.int16
AF = mybir.ActivationFunctionType
ALU = mybir.AluOpType

NP_FP8 = ml_dtypes.float8_e3m4
NP_BF16 = ml_dtypes.bfloat16

MAXNORM = 1.0 - 1e-5
MIN_SS = 1e-15
XSCALE = 8.0     # x shipped as x*XSCALE in fp8, W1 shipped as W1/XSCALE
S2SCALE = 4.0    # s2 stored as s2*S2SCALE in fp8 (undone in the L2 tanh)

GRP = 8          # dst blocks per L2 PSUM group
WB = 8           # windows per L2 gather call (1024-idx hw ring limit)
WB1 = 32         # windows per L1 stream DMA
IB = 8           # gather calls per idx DMA
SB = 4           # s2 blocks per spill DMA
NQ = 4           # exchange stages / L2 chunks / sub-tables


class Cfg:
    def __init__(self, n_nodes, in_dim, hid, n_seg, n_cores):
        self.N = n_nodes
        self.IN = in_dim
        self.HID = hid
        self.NSEG = n_seg
        self.NC = n_cores
        self.SHARD = n_nodes // n_cores
        self.NBLK = self.SHARD // 128
        self.NGRP = self.NBLK // GRP
        self.NSEGCH = (n_seg + 127) // 128
        self.QROWS = self.SHARD // NQ            # rows per core per stage
        self.SUBROWS = self.NC * self.QROWS      # rows per sub-table (32768)
        self.QBLK = self.NBLK // NQ              # L1 blocks per stage


def _prep_l1(cfg, src, dst):
    """Per-block windows (1 block per window). Streams: srcidx / slot
    (pad: srcidx=-1, slot=-1)."""
    NC, NBLK = cfg.NC, cfg.NBLK
    core = dst // cfg.SHARD
    nb = (dst % cfg.SHARD) // 128
    slot = dst % 128
    cnt = np.zeros((NC, NBLK), dtype=np.int64)
    np.add.at(cnt, (core, nb), 1)
    nw1 = (cnt.max(axis=0) + 127) // 128
    nw1 = np.maximum(nw1, 1)
    base1 = np.concatenate([[0], np.cumsum(nw1)[:-1]])
    NW1 = int(nw1.sum())
    TOT = NW1 * 128
    order = np.lexsort((slot, nb, core))
    per_core = []
    for c in range(NC):
        sel = order[core[order] == c]
        sidx = np.full(TOT, -1, dtype=np.int64)
        sslot = np.full(TOT, -1, dtype=np.int64)
        ep = 0
        for b in range(NBLK):
            n = int(cnt[c, b])
            pos = int(base1[b]) * 128
            if n:
                s = sel[ep:ep + n]
                sidx[pos:pos + n] = src[s]
                sslot[pos:pos + n] = slot[s]
                ep += n
        assert ep == len(sel)
        per_core.append({"srcidx": sidx, "slot": sslot})
    return {"nw1": nw1, "base1": base1, "NW1": NW1, "per_core": per_core}


def _prep_l2(cfg, src, dst):
    """Cells (g, t): t = quarter of the src shard. Windows per cell padded
    to max over cores; entries per (window, b); per-t contiguous streams."""
    NC, NGRP = cfg.NC, cfg.NGRP
    core = dst // cfg.SHARD
    blk = (dst % cfg.SHARD) // 128
    slot = dst % 128
    g_all = blk // GRP
    b_all = blk % GRP
    t_all = (src % cfg.SHARD) // cfg.QROWS
    pos_sub = (src // cfg.SHARD) * cfg.QROWS + (src % cfg.QROWS)

    cnt = np.zeros((NC, NGRP, NQ), dtype=np.int64)
    np.add.at(cnt, (core, g_all, t_all), 1)
    nw2 = (cnt.max(axis=0) + 127) // 128      # [NGRP, NQ]
    nw2 = np.maximum(nw2, 1)                  # every (g, t) cell exists

    # per-t stream window bases, consumption order (t, g)
    wbase = np.zeros((NGRP, NQ), dtype=np.int64)
    nwt = np.zeros(NQ, dtype=np.int64)
    for t in range(NQ):
        for g in range(NGRP):
            wbase[g, t] = nwt[t]
            nwt[t] += nw2[g, t]

    order = np.lexsort((b_all, g_all, t_all, core))
    per_core = []
    for c in range(NC):
        sel = order[core[order] == c]
        streams_idx = [np.zeros(int(nwt[t]) * 128, dtype=np.int64) for t in range(NQ)]
        streams_slot = [np.full(int(nwt[t]) * 128, -1, dtype=np.int64) for t in range(NQ)]
        streams_blk = [np.full(int(nwt[t]) * 128, -1, dtype=np.int64) for t in range(NQ)]
        ep = 0
        for t in range(NQ):
            for g in range(NGRP):
                n = int(cnt[c, g, t])
                if n == 0:
                    continue
                s = sel[ep:ep + n]
                pos = int(wbase[g, t]) * 128
                streams_idx[t][pos:pos + n] = pos_sub[s]
                streams_slot[t][pos:pos + n] = slot[s]
                streams_blk[t][pos:pos + n] = b_all[s]
                ep += n
        assert ep == len(sel)
        per_core.append({"idx": streams_idx, "slot": streams_slot,
                         "blk": streams_blk})

    # entries: union over cores of (t-window, b) touches; force every
    # (g, b, t) to have >= 1 entry (sacc init at t=0, finalize at t=NQ-1)
    touched = [np.zeros((int(nwt[t]), GRP), dtype=bool) for t in range(NQ)]
    for c in range(NC):
        for t in range(NQ):
            sb = per_core[c]["blk"][t].reshape(-1, 128)
            for b in range(GRP):
                touched[t][:, b] |= (sb == b).any(axis=1)
    for g in range(NGRP):
        for t in range(NQ):
            w0, n = int(wbase[g, t]), int(nw2[g, t])
            for b in range(GRP):
                if not touched[t][w0:w0 + n, b].any():
                    touched[t][w0, b] = True

    # entry ids in consumption order (t, g, w, b) + start/stop per (g,b,t)
    sched = []   # sched[t][g] = list of (wg, [(ent, b, st, sp)])
    nent = 0
    for t in range(NQ):
        st_g = []
        for g in range(NGRP):
            cellw = []
            went = {}
            w0, n = int(wbase[g, t]), int(nw2[g, t])
            for lw in range(n):
                ents = []
                for b in range(GRP):
                    if touched[t][w0 + lw, b]:
                        ents.append([nent, b, False, False])
                        went.setdefault(b, []).append((len(cellw), len(ents) - 1))
                        nent += 1
                cellw.append((w0 + lw, ents))
            for b, lst in went.items():
                wi, ei = lst[0]
                cellw[wi][1][ei][2] = True
                wi, ei = lst[-1]
                cellw[wi][1][ei][3] = True
            st_g.append(cellw)
        sched.append(st_g)

    for c in range(NC):
        scol = np.full((nent, 128), -1.0, dtype=np.float32)
        for t in range(NQ):
            for g in range(NGRP):
                for (wg, ents) in sched[t][g]:
                    sb = per_core[c]["blk"][t][wg * 128:(wg + 1) * 128]
                    ss_ = per_core[c]["slot"][t][wg * 128:(wg + 1) * 128]
                    for (ent, b, _, _) in ents:
                        scol[ent] = np.where(sb == b, ss_, -1).astype(np.float32)
        per_core[c]["slotcol"] = scol

    return {"nw2": nw2, "wbase": wbase, "nwt": nwt, "sched": sched,
            "nent": nent, "per_core": per_core}


def host_prep(cfg, src, dst):
    src = np.asarray(src).astype(np.int64)
    dst = np.asarray(dst).astype(np.int64)
    l1 = _prep_l1(cfg, src, dst)
    l2 = _prep_l2(cfg, src, dst)
    return l1, l2


def build(cfg, l1, l2):
    N, IN, HID = cfg.N, cfg.IN, cfg.HID
    NW1 = l1["NW1"]
    nwt = [int(x) for x in l2["nwt"]]
    NENT2 = l2["nent"]

    nc = bacc.Bacc("TRN2", target_bir_lowering=False)

    xs_d = nc.dram_tensor("xs", [128, NW1 * IN], FP8, kind="ExternalInput")
    slot1_d = nc.dram_tensor("slot1", [128, NW1], F32, kind="ExternalInput")
    idx_d = [nc.dram_tensor(f"idx{t}", [128, max(nwt[t] * 8, 8)], I16,
                            kind="ExternalInput") for t in range(NQ)]
    slot2_d = nc.dram_tensor("slot2", [128, NENT2], F32, kind="ExternalInput")
    segid_d = nc.dram_tensor("segid", [128, cfg.NBLK], F32, kind="ExternalInput")
    iota_d = nc.dram_tensor("iota128", [128, 128], BF16, kind="ExternalInput")
    iotas_d = nc.dram_tensor("iota_seg", [128, cfg.NSEGCH * 128], FP16, kind="ExternalInput")
    ident_d = nc.dram_tensor("ident", [128, 128], BF16, kind="ExternalInput")
    identf_d = nc.dram_tensor("identf", [128, 128], FP8, kind="ExternalInput")
    ident32_d = nc.dram_tensor("ident32", [128, 128], F32, kind="ExternalInput")
    w1_d = nc.dram_tensor("W1s", [IN, HID], BF16, kind="ExternalInput")
    w2_d = nc.dram_tensor("W2s", [HID, HID], BF16, kind="ExternalInput")
    b1_d = nc.dram_tensor("b1col", [128, 1], F32, kind="ExternalInput")
    b2_d = nc.dram_tensor("b2col", [128, 1], F32, kind="ExternalInput")
    ones_d = nc.dram_tensor("onescol", [128, 1], BF16, kind="ExternalInput")

    s2_sh = [nc.dram_tensor(f"s2_sh{q}", [cfg.QROWS, HID], FP8)
             for q in range(NQ)]
    s2_full = [nc.dram_tensor(f"s2_full{q}", [cfg.SUBROWS, HID], FP8,
                              addr_space="Shared") for q in range(NQ)]
    s2_pad = [nc.dram_tensor(f"s2_pad{q}", [cfg.SUBROWS, 2 * HID], FP8)
              for q in range(NQ)]
    out_d = nc.dram_tensor("pooled", [cfg.NSEGCH * 128, HID + 1], F32,
                           kind="ExternalOutput")

    KIN = IN // 128
    nw1 = [int(x) for x in l1["nw1"]]
    base1 = [int(x) for x in l1["base1"]]

    with tile.TileContext(nc) as tc, ExitStack() as ctx:
        const = ctx.enter_context(tc.tile_pool(name="const", bufs=1))
        xsp = ctx.enter_context(tc.tile_pool(name="xsp", bufs=3))
        sp = ctx.enter_context(tc.tile_pool(name="sp", bufs=20))
        hp = ctx.enter_context(tc.tile_pool(name="hp", bufs=3))
        saccp = ctx.enter_context(tc.tile_pool(name="saccp", bufs=1))
        normp = ctx.enter_context(tc.tile_pool(name="normp", bufs=1))

        # ---- constants ----
        iota128 = const.tile([128, 128], BF16)
        nc.sync.dma_start(iota128[:], iota_d[:])
        iotaseg = const.tile([128, cfg.NSEGCH * 128], FP16)
        nc.sync.dma_start(iotaseg[:], iotas_d[:])
        ident = const.tile([128, 128], BF16)
        nc.sync.dma_start(ident[:], ident_d[:])
        identf = const.tile([128, 128], FP8)
        nc.sync.dma_start(identf[:], identf_d[:])
        ident32 = const.tile([128, 128], F32)
        nc.sync.dma_start(ident32[:], ident32_d[:])
        segid = const.tile([128, cfg.NBLK], F32)
        nc.sync.dma_start(segid[:], segid_d[:])
        w1_sb = [const.tile([128, HID], BF16, tag=f"w1_{k}", name=f"w1_{k}")
                 for k in range(KIN)]
        for k in range(KIN):
            nc.sync.dma_start(w1_sb[k][:], w1_d[k * 128:(k + 1) * 128, :])
        w2_sb = const.tile([128, HID], BF16)
        nc.sync.dma_start(w2_sb[:], w2_d[:])
        b1c = const.tile([128, 1], F32)
        nc.sync.dma_start(b1c[:], b1_d[:])
        b2c = const.tile([128, 1], F32)
        nc.sync.dma_start(b2c[:], b2_d[:])
        onesc = const.tile([128, 1], BF16)
        nc.sync.dma_start(onesc[:], ones_d[:])
        slot1 = const.tile([128, NW1], F32)
        nc.sync.dma_start(slot1[:], slot1_d[:])
        slot2 = const.tile([128, NENT2], F32)
        nc.sync.dma_start(slot2[:], slot2_d[:])

        # ================= layer 1 =================
        ctx_l1 = ctx.enter_context(ExitStack())
        ps_acc = ctx_l1.enter_context(tc.tile_pool(name="ps_acc", bufs=3, space="PSUM"))
        ps_h = ctx_l1.enter_context(tc.tile_pool(name="ps_h", bufs=1, space="PSUM"))
        ps_s2 = ctx_l1.enter_context(tc.tile_pool(name="ps_s2", bufs=1, space="PSUM"))

        s2n_cur = [None]

        def l1_block(nb, acc):
            # acc: PSUM [128, 256] f32 FEATURE-major (aggT chunk k at cols
            # k*128..): no PE transposes anywhere in L1 (transposes are
            # serialized against in-flight collectives by the tile layer).
            h_ps = ps_h.tile([128, 128], F32, tag="hps", name="h_ps")
            for k in range(KIN):
                xt = hp.tile([128, 128], BF16, tag="xt", name="xt")
                nc.scalar.activation(xt[:], acc[:, k * 128:(k + 1) * 128], AF.Copy)
                nc.tensor.matmul(h_ps[:], w1_sb[k][:], xt[:],
                                 start=(k == 0), stop=(k == KIN - 1))
            h1t = hp.tile([128, 128], BF16, tag="h1t", name="h1t")
            nc.scalar.activation(h1t[:], h_ps[:], AF.Tanh, bias=b1c[:, 0:1])
            # node-major s2 directly: out[slot, hid] = h1T^T @ W2
            s2_ps = ps_s2.tile([128, 128], F32, tag="s2ps", name="s2_ps")
            nc.tensor.matmul(s2_ps[:], h1t[:], w2_sb[:], start=True, stop=True)
            jb = nb % SB
            if jb == 0:
                s2n_cur[0] = hp.tile([128, SB * 128], FP8, tag="s2n", name="s2n")
            s2n = s2n_cur[0]
            nc.scalar.activation(s2n[:, jb * 128:(jb + 1) * 128], s2_ps[:], AF.Copy)
            if jb == SB - 1:
                q = nb // cfg.QBLK
                r0 = ((nb - SB + 1) % cfg.QBLK) * 128
                nc.sync.dma_start(
                    s2_sh[q][r0:r0 + SB * 128, :].rearrange(
                        "(j p) f -> p j f", j=SB),
                    s2n[:].rearrange("p (j f) -> p j f", f=128))

        def expand(q):
            ER = 8192   # expand rows per DMA (SWDGE <16384-desc limit)
            for e0 in range(0, cfg.SUBROWS, ER):
                nc.gpsimd.dma_start(s2_pad[q][e0:e0 + ER, 0:HID],
                                    s2_full[q][e0:e0 + ER, :])

        win_blk = []
        for b in range(cfg.NBLK):
            win_blk += [b] * nw1[b]
        acc_cur = {}
        for w0 in range(0, NW1, WB1):
            nwb = min(WB1, NW1 - w0)
            eb = xsp.tile([128, WB1 * IN], FP8, tag="xs", name="xs")
            nc.sync.dma_start(eb[:, :nwb * IN],
                              xs_d[:, w0 * IN:(w0 + nwb) * IN])
            m1s = []
            for j in range(nwb):
                w = w0 + j
                m1 = sp.tile([128, 128], BF16, tag="m1", name="m1")
                nc.vector.tensor_scalar(m1[:], iota128[:], slot1[:, w:w + 1],
                                        None, ALU.is_equal)
                m1s.append(m1)
            for j in range(nwb):
                w = w0 + j
                b = win_blk[w]
                if b not in acc_cur:
                    acc_cur[b] = ps_acc.tile([128, KIN * 128], F32, tag="acc",
                                             name="acc")
                acc = acc_cur[b]
                for k in range(KIN):
                    nc.tensor.matmul(
                        acc[:, k * 128:(k + 1) * 128],
                        eb[:, j * IN + k * 128:j * IN + (k + 1) * 128],
                        m1s[j][:],
                        start=(w == base1[b]),
                        stop=(w == base1[b] + nw1[b] - 1))
                if w == base1[b] + nw1[b] - 1:
                    l1_block(b, acc)
                    del acc_cur[b]
                    if (b + 1) % cfg.QBLK == 0:
                        q = b // cfg.QBLK
                        nc.gpsimd.collective_compute(
                            "AllGather", ALU.bypass,
                            ins=[s2_sh[q].ap().opt()],
                            outs=[s2_full[q].ap().opt()],
                            replica_groups=[list(range(cfg.NC))])
                        # expand table q-1 (its AG is done by now, so the
                        # SEQ wait is short and never blocks gathers)
                        if q >= 1:
                            expand(q - 1)

        # ================= layer 2 =================
        ctx_l1.close()
        ctx_l2 = ctx.enter_context(ExitStack())
        pacc_p = ctx_l2.enter_context(tc.tile_pool(name="pacc", bufs=2, space="PSUM"))
        ps_pool = ctx_l2.enter_context(tc.tile_pool(name="ps_pool", bufs=1, space="PSUM"))
        ps_ss = ctx_l2.enter_context(tc.tile_pool(name="ps_ss", bufs=1, space="PSUM"))
        ps_h2 = ctx_l2.enter_context(tc.tile_pool(name="ps_h2", bufs=2, space="PSUM"))
        ebp = ctx_l2.enter_context(tc.tile_pool(name="ebp", bufs=6))
        idxp = ctx_l2.enter_context(tc.tile_pool(name="idxp", bufs=3))
        htp = ctx_l2.enter_context(tc.tile_pool(name="htp", bufs=4))

        sacc = saccp.tile([128, cfg.NBLK * 128], F32)
        h2_all = saccp.tile([128, cfg.NBLK * 128], BF16, name="h2_all")
        ss_all = ps_ss.tile([128, cfg.NBLK], F32, name="ss_all")
        scale = normp.tile([128, cfg.NBLK], F32)
        pool_all = ps_pool.tile([128, cfg.NSEGCH * (HID + 1)], F32, name="pool_all")
        pool_ps = [pool_all[:, s * (HID + 1):(s + 1) * (HID + 1)]
                   for s in range(cfg.NSEGCH)]

        cur_tile = [None] * NQ
        cur_w0 = [0] * NQ
        cur_it = [None] * NQ
        cur_it_w0 = [-1] * NQ

        def ensure_window(t, wg):
            if cur_tile[t] is None or wg >= cur_w0[t] + WB:
                w0 = (wg // WB) * WB
                iw0 = (w0 // (WB * IB)) * (WB * IB)
                if cur_it_w0[t] != iw0:
                    niw = min(WB * IB, nwt[t] - iw0)
                    it = idxp.tile([128, WB * IB * 8], I16, tag="it", name="it")
                    nc.sync.dma_start(it[:, :niw * 8],
                                      idx_d[t][:, iw0 * 8:(iw0 + niw) * 8])
                    cur_it[t] = it
                    cur_it_w0[t] = iw0
                nwin = min(WB, nwt[t] - w0)
                jo = (w0 - iw0) * 8
                eb = ebp.tile([128, WB * 2 * HID], FP8, tag="eb", name="eb")
                nc.gpsimd.dma_gather(
                    out_ap=eb[:, :nwin * 2 * HID].rearrange(
                        "p (n f) -> p n f", f=2 * HID),
                    in_ap=s2_pad[t][:, :],
                    idxs_ap=cur_it[t][:, jo:jo + nwin * 8],
                    num_idxs=nwin * 128,
                    num_idxs_reg=nwin * 128,
                    elem_size=2 * HID)
                cur_tile[t] = eb
                cur_w0[t] = w0
            return cur_tile[t], wg - cur_w0[t]

        FLUSH_AT = (32, 64, 96, 128)
        h2_pend = []

        def flush_logmap():
            if not h2_pend:
                return
            b0 = h2_pend[0]
            nbk = len(h2_pend)
            ss = ss_all[:, b0:b0 + nbk]
            na = normp.tile([128, 32], F32, tag="na", name="na")
            nc.vector.tensor_scalar_max(na[:, :nbk], ss, MIN_SS)
            nrm = normp.tile([128, 32], F32, tag="nrm", name="nrm")
            nc.scalar.activation(nrm[:, :nbk], na[:, :nbk], AF.Sqrt)
            ncl = normp.tile([128, 32], F32, tag="ncl", name="ncl")
            nc.vector.tensor_scalar_min(ncl[:, :nbk], nrm[:, :nbk], MAXNORM)
            om = normp.tile([128, 32], F32, tag="om", name="om")
            nc.vector.tensor_scalar(om[:, :nbk], ncl[:, :nbk], -1.0, 1.0,
                                    ALU.mult, ALU.add)
            op_ = normp.tile([128, 32], F32, tag="op", name="op_")
            nc.vector.tensor_scalar_add(op_[:, :nbk], ncl[:, :nbk], 1.0)
            rc = normp.tile([128, 32], F32, tag="rc", name="rc")
            nc.vector.reciprocal(rc[:, :nbk], om[:, :nbk])
            rat = normp.tile([128, 32], F32, tag="rat", name="rat")
            nc.vector.tensor_mul(rat[:, :nbk], op_[:, :nbk], rc[:, :nbk])
            lg = normp.tile([128, 32], F32, tag="lg", name="lg")
            nc.scalar.activation(lg[:, :nbk], rat[:, :nbk], AF.Ln)
            rcn = normp.tile([128, 32], F32, tag="rcn", name="rcn")
            nc.vector.reciprocal(rcn[:, :nbk], nrm[:, :nbk])
            nc.vector.tensor_mul(rcn[:, :nbk], rcn[:, :nbk], lg[:, :nbk])
            nc.vector.tensor_scalar_mul(scale[:, b0:b0 + nbk], rcn[:, :nbk], 0.5)
            for nb in h2_pend:
                ht = htp.tile([128, HID + 1], BF16, tag="ht", name="ht")
                nc.vector.tensor_scalar(ht[:, :HID],
                                        h2_all[:, nb * 128:(nb + 1) * 128],
                                        scale[:, nb:nb + 1], None, ALU.mult)
                nc.vector.memset(ht[:, HID:HID + 1], 1.0)
                sg = sp.tile([128, cfg.NSEGCH * 128], FP16, tag="sg", name="sg")
                nc.vector.tensor_scalar(sg[:], iotaseg[:], segid[:, nb:nb + 1],
                                        None, ALU.is_equal)
                for s in range(cfg.NSEGCH):
                    nc.tensor.matmul(pool_ps[s], sg[:, s * 128:(s + 1) * 128],
                                     ht[:], start=(nb == 0),
                                     stop=(nb == cfg.NBLK - 1))
            h2_pend.clear()

        def l2_flush(t, g, pacc):
            for b in range(GRP):
                nb = g * GRP + b
                pb = pacc[:, b * 128:(b + 1) * 128]
                sl = sacc[:, nb * 128:(nb + 1) * 128]
                if t < NQ - 1:
                    nc.scalar.activation(sl, pb, AF.Copy)
                else:
                    h2t = hp.tile([128, 128], BF16, tag="h2t", name="h2t")
                    nc.scalar.activation(h2t[:], pb, AF.Tanh,
                                         bias=b2c[:, 0:1],
                                         scale=1.0 / S2SCALE)
                    sq = hp.tile([128, 128], BF16, tag="sq", name="sq")
                    nc.scalar.activation(sq[:], h2t[:], AF.Square)
                    nc.tensor.matmul(ss_all[:, nb:nb + 1], sq[:], onesc[:],
                                     start=True, stop=True)
                    h2tr = ps_h2.tile([128, 128], BF16, tag="h2b",
                                      name="h2tr")
                    nc.tensor.transpose(h2tr[:], h2t[:], ident[:])
                    nc.scalar.activation(
                        h2_all[:, nb * 128:(nb + 1) * 128], h2tr[:], AF.Copy)
                    h2_pend.append(nb)
                    if nb + 1 in FLUSH_AT:
                        flush_logmap()

        for t in range(NQ):
            # expand table t+1 after stage t-1's gathers are all emitted:
            # Pool order [.. g_{t-1} .., exp_{t+1}, g_t ..] so the expand's
            # collective wait never delays an earlier stage's gathers.
            if 2 <= t + 1 < NQ:
                expand(t + 1)
            # schedule as a flat list of (g, wg, ents); batch masks per
            # gather call (WB windows) ahead of the matmuls
            flat = []
            for g in range(cfg.NGRP):
                for (wg, ents) in l2["sched"][t][g]:
                    flat.append((g, wg, ents))
            gleft = {g: sum(len(e) for (_, e) in l2["sched"][t][g])
                     for g in range(cfg.NGRP)}
            paccs = {}
            for c0 in range(0, len(flat), WB):
                chunk = flat[c0:c0 + WB]
                eb, _ = ensure_window(t, chunk[0][1])
                masks = {}
                for (g, wg, ents) in chunk:
                    for (ent, b, st_f, sp_f) in ents:
                        m = sp.tile([128, 128], BF16, tag="m2", name="m2")
                        nc.vector.tensor_scalar(m[:], iota128[:],
                                                slot2[:, ent:ent + 1],
                                                None, ALU.is_equal)
                        masks[ent] = m
                for (g, wg, ents) in chunk:
                    eb, joff = ensure_window(t, wg)
                    if g not in paccs:
                        paccs[g] = pacc_p.tile([128, GRP * 128], F32,
                                               tag="pacc", name="pacc")
                    for (ent, b, st_f, sp_f) in ents:
                        if st_f and t > 0:
                            # seed the PSUM acc with the running partial
                            nc.tensor.matmul(
                                paccs[g][:, b * 128:(b + 1) * 128],
                                ident32[:],
                                sacc[:, (g * GRP + b) * 128:
                                     (g * GRP + b + 1) * 128],
                                start=True, stop=False)
                            st_f = False
                        nc.tensor.matmul(
                            paccs[g][:, b * 128:(b + 1) * 128],
                            eb[:, joff * 2 * HID:joff * 2 * HID + HID],
                            masks[ent][:], start=st_f, stop=sp_f)
                        gleft[g] -= 1
                        if gleft[g] == 0:
                            l2_flush(t, g, paccs.pop(g))
        flush_logmap()

        for s in range(cfg.NSEGCH):
            po = htp.tile([128, HID + 1], F32, tag="po", name="po")
            nc.vector.tensor_copy(po[:], pool_ps[s])
            nc.sync.dma_start(out_d[s * 128:(s + 1) * 128, :], po[:])

    nc.compile()
    return nc


def host_inputs(cfg, x, seg_ids, W1, b1, W2, b2, l1, l2):
    N, IN, HID = cfg.N, cfg.IN, cfg.HID
    NW1 = l1["NW1"]
    xb = np.ascontiguousarray(
        (np.asarray(x, np.float32) * XSCALE).astype(NP_FP8))
    iota128 = np.tile(np.arange(128, dtype=np.float32), (128, 1)).astype(NP_BF16)
    iotaseg = np.tile(np.arange(cfg.NSEGCH * 128, dtype=np.float32),
                      (128, 1)).astype(np.float16)
    ident = np.eye(128, dtype=np.float32).astype(NP_BF16)
    identf = np.eye(128, dtype=np.float32).astype(NP_FP8)
    w1 = np.ascontiguousarray(
        (np.asarray(W1, np.float32) / XSCALE).astype(NP_BF16))
    w2 = np.ascontiguousarray(
        (np.asarray(W2, np.float32) * S2SCALE).astype(NP_BF16))
    b1c = np.asarray(b1, np.float32).reshape(128, 1)
    b2c = np.asarray(b2, np.float32).reshape(128, 1)
    ones = np.ones((128, 1), np.float32).astype(NP_BF16)
    seg = np.asarray(seg_ids, np.float32)

    maps = []
    for c in range(cfg.NC):
        pc1 = l1["per_core"][c]
        sidx = pc1["srcidx"]
        rows = xb[np.maximum(sidx, 0)]                     # [NW1*128, 256]
        rows[sidx < 0] = 0
        xs = np.ascontiguousarray(
            rows.reshape(NW1, 128, IN).transpose(1, 0, 2).reshape(128, NW1 * IN))
        slot1 = np.ascontiguousarray(
            pc1["slot"].reshape(NW1, 128).T.astype(np.float32))

        pc2 = l2["per_core"][c]
        idxs = {}
        for t in range(NQ):
            ids = pc2["idx"][t]
            iw = ids.astype(np.int16).reshape(-1, 16).T
            iw = np.tile(iw, (8, 1)).copy()
            idxs[f"idx{t}"] = np.ascontiguousarray(iw.astype(np.int16))

        segc = seg[c * cfg.SHARD:(c + 1) * cfg.SHARD].reshape(cfg.NBLK, 128).T
        maps.append({
            "xs": xs,
            "slot1": slot1,
            **idxs,
            "slot2": np.ascontiguousarray(pc2["slotcol"].T),
            "segid": np.ascontiguousarray(segc.astype(np.float32)),
            "iota128": iota128,
            "iota_seg": iotaseg,
            "ident": ident,
            "identf": identf,
            "ident32": np.eye(128, dtype=np.float32),
            "W1s": w1,
            "W2s": w2,
            "b1col": b1c,
            "b2col": b2c,
            "onescol": ones,
        })
    return maps


def host_epilogue(cfg, partials, batch_size, max_comments):
    acc = np.zeros_like(partials[0], dtype=np.float64)
    for p in partials:
        acc += p.astype(np.float64)
    acc = acc.astype(np.float32)
    nseg = cfg.NSEG
    sums = acc[:nseg, :cfg.HID]
    counts = acc[:nseg, cfg.HID]
    agg = sums / np.maximum(counts, 1.0)[:, None]
    ss = np.maximum(np.sum(agg * agg, axis=1), MIN_SS).astype(np.float32)
    norm = np.sqrt(ss)
    y = agg * (np.tanh(norm) / norm)[:, None]
    ssy = np.maximum(np.sum(y * y, axis=1), MIN_SS).astype(np.float32)
    ny = np.sqrt(ssy)
    f = np.where(ny > MAXNORM, MAXNORM / ny, 1.0).astype(np.float32)
    y = y * f[:, None]
    return y.reshape(int(batch_size), int(max_comments), cfg.HID)


# ====================================================================
# Harness entry point
# ====================================================================

_CACHE = {}


def kernel(x, src, dst, seg_ids, W1, b1, W2, b2, batch_size, max_comments):
    """Full-input GNN ComEnc kernel on 8 Trainium2 NeuronCores."""
    from concourse.bass_utils import run_bass_kernel_spmd

    x = np.asarray(x, dtype=np.float32)
    src = np.asarray(src).astype(np.int64)
    dst = np.asarray(dst).astype(np.int64)
    seg_ids = np.asarray(seg_ids).astype(np.int64)
    W1 = np.asarray(W1, dtype=np.float32)
    b1 = np.asarray(b1, dtype=np.float32)
    W2 = np.asarray(W2, dtype=np.float32)
    b2 = np.asarray(b2, dtype=np.float32)
    bs = int(np.asarray(batch_size))
    mc = int(np.asarray(max_comments))

    n_nodes, in_dim = x.shape
    hid = W1.shape[1]
    nseg = bs * mc
    n_cores = 8

    cfg = Cfg(n_nodes, in_dim, hid, nseg, n_cores)
    l1, l2 = host_prep(cfg, src, dst)

    key = (n_nodes, in_dim, hid, nseg, l1["NW1"], l2["nent"],
           tuple(int(v) for v in l2["nwt"]))
    if key in _CACHE:
        nc = _CACHE[key]
    else:
        nc = build(cfg, l1, l2)
        _CACHE.clear()
        _CACHE[key] = nc

    maps = host_inputs(cfg, x, seg_ids, W1, b1, W2, b2, l1, l2)
    res = run_bass_kernel_spmd(nc, maps, core_ids=list(range(n_cores)))
    partials = [r["pooled"] for r in res.results]
    out = host_epilogue(cfg, partials, bs, mc)
    return np.ascontiguousarray(out.astype(np.float32))


# revision 18
# speedup vs baseline: 1.0179x; 1.0179x over previous
"""GNN message-passing kernel for Trainium2 (8 NeuronCores, SPMD) — v3.

Computation (see reference):
  h1 = tanh(A x @ W1 + b1)          [A(xW) = (Ax)W]
  s2 = h1 @ W2
  h2 = tanh(A s2 + b2)
  ht = logmap0(proj(h2))            (rowwise scale)
  pooled = segment mean over seg_ids, then expmap0/proj (host epilogue)

Sharding: nodes split contiguously over cores (dst-shard), SHARD=16384.

v3 structure per core:
  L1  streams host-pregathered x rows (bf16) in per-block windows (one
      128-edge window per dst block, pads masked).  One DVE is_equal
      one-hot per window feeds one PE matmul into the node-major block
      acc [slot, 256].  Epilogue transposes the acc, h1T = W1^T accT
      (Act tanh with per-partition bias), s2T = (4*W2)^T h1T, transpose
      to node-major fp8e3 s2 rows (scaled x4 for fp8 range).
  Exchange: FOUR fp8 AllGathers (one per quarter of the node space,
      issued as soon as that quarter's L1 blocks finish), each expanded
      by a DRAM->DRAM DMA into a 256B-strided padded table so dma_gather
      can fetch 256-byte descriptors legally.
  L2  four chunks (= exchange stages = int16 sub-tables).  Contiguous
      per-table gather streams (8x128 rows per call, batched idx loads),
      DVE slot masks, feature-major accumulation (lhsT = gathered fp8
      rows), PSUM acc per (group,block) within a chunk, SBUF partial
      across chunks; finalize = add + Act tanh(scale=0.25, bias) ->
      norms via PE ones-matmul -> batched logmap scale -> transpose ->
      pooling via fp16 seg masks.
"""

import numpy as np
import ml_dtypes
from contextlib import ExitStack

import concourse.bass as bass
import concourse.tile as tile
import concourse.bacc as bacc
from concourse import mybir

BF16 = mybir.dt.bfloat16
FP16 = mybir.dt.float16
FP8 = mybir.dt.float8e3
F32 = mybir.dt.float32
I16 = mybir.dt.int16
AF = mybir.ActivationFunctionType
ALU = mybir.AluOpType

NP_FP8 = ml_dtypes.float8_e3m4
NP_BF16 = ml_dtypes.bfloat16

MAXNORM = 1.0 - 1e-5
MIN_SS = 1e-15
XSCALE = 8.0     # x shipped as x*XSCALE in fp8, W1 shipped as W1/XSCALE
S2SCALE = 4.0    # s2 stored as s2*S2SCALE in fp8 (undone in the L2 tanh)

GRP = 8          # dst blocks per L2 PSUM group
WB = 8           # windows per L2 gather call (1024-idx hw ring limit)
WB1 = 32         # windows per L1 stream DMA
IB = 8           # gather calls per idx DMA
SB = 4           # s2 blocks per spill DMA
NQ = 4           # exchange stages / L2 chunks / sub-tables


class Cfg:
    def __init__(self, n_nodes, in_dim, hid, n_seg, n_cores):
        self.N = n_nodes
        self.IN = in_dim
        self.HID = hid
        self.NSEG = n_seg
        self.NC = n_cores
        self.SHARD = n_nodes // n_cores
        self.NBLK = self.SHARD // 128
        self.NGRP = self.NBLK // GRP
        self.NSEGCH = (n_seg + 127) // 128
        self.QROWS = self.SHARD // NQ            # rows per core per stage
        self.SUBROWS = self.NC * self.QROWS      # rows per sub-table (32768)
        self.QBLK = self.NBLK // NQ              # L1 blocks per stage


def _prep_l1(cfg, src, dst):
    """Per-block windows (1 block per window). Streams: srcidx / slot
    (pad: srcidx=-1, slot=-1)."""
    NC, NBLK = cfg.NC, cfg.NBLK
    core = dst // cfg.SHARD
    nb = (dst % cfg.SHARD) // 128
    slot = dst % 128
    cnt = np.zeros((NC, NBLK), dtype=np.int64)
    np.add.at(cnt, (core, nb), 1)
    nw1 = (cnt.max(axis=0) + 127) // 128
    nw1 = np.maximum(nw1, 1)
    base1 = np.concatenate([[0], np.cumsum(nw1)[:-1]])
    NW1 = int(nw1.sum())
    TOT = NW1 * 128
    order = np.lexsort((slot, nb, core))
    per_core = []
    for c in range(NC):
        sel = order[core[order] == c]
        sidx = np.full(TOT, -1, dtype=np.int64)
        sslot = np.full(TOT, -1, dtype=np.int64)
        ep = 0
        for b in range(NBLK):
            n = int(cnt[c, b])
            pos = int(base1[b]) * 128
            if n:
                s = sel[ep:ep + n]
                sidx[pos:pos + n] = src[s]
                sslot[pos:pos + n] = slot[s]
                ep += n
        assert ep == len(sel)
        per_core.append({"srcidx": sidx, "slot": sslot})
    return {"nw1": nw1, "base1": base1, "NW1": NW1, "per_core": per_core}


def _prep_l2(cfg, src, dst):
    """Cells (g, t): t = quarter of the src shard. Windows per cell padded
    to max over cores; entries per (window, b); per-t contiguous streams."""
    NC, NGRP = cfg.NC, cfg.NGRP
    core = dst // cfg.SHARD
    blk = (dst % cfg.SHARD) // 128
    slot = dst % 128
    g_all = blk // GRP
    b_all = blk % GRP
    t_all = (src % cfg.SHARD) // cfg.QROWS
    pos_sub = (src // cfg.SHARD) * cfg.QROWS + (src % cfg.QROWS)

    cnt = np.zeros((NC, NGRP, NQ), dtype=np.int64)
    np.add.at(cnt, (core, g_all, t_all), 1)
    nw2 = (cnt.max(axis=0) + 127) // 128      # [NGRP, NQ]
    nw2 = np.maximum(nw2, 1)                  # every (g, t) cell exists

    # per-t stream window bases, consumption order (t, g)
    wbase = np.zeros((NGRP, NQ), dtype=np.int64)
    nwt = np.zeros(NQ, dtype=np.int64)
    for t in range(NQ):
        for g in range(NGRP):
            wbase[g, t] = nwt[t]
            nwt[t] += nw2[g, t]

    order = np.lexsort((b_all, g_all, t_all, core))
    per_core = []
    for c in range(NC):
        sel = order[core[order] == c]
        streams_idx = [np.zeros(int(nwt[t]) * 128, dtype=np.int64) for t in range(NQ)]
        streams_slot = [np.full(int(nwt[t]) * 128, -1, dtype=np.int64) for t in range(NQ)]
        streams_blk = [np.full(int(nwt[t]) * 128, -1, dtype=np.int64) for t in range(NQ)]
        ep = 0
        for t in range(NQ):
            for g in range(NGRP):
                n = int(cnt[c, g, t])
                if n == 0:
                    continue
                s = sel[ep:ep + n]
                pos = int(wbase[g, t]) * 128
                streams_idx[t][pos:pos + n] = pos_sub[s]
                streams_slot[t][pos:pos + n] = slot[s]
                streams_blk[t][pos:pos + n] = b_all[s]
                ep += n
        assert ep == len(sel)
        per_core.append({"idx": streams_idx, "slot": streams_slot,
                         "blk": streams_blk})

    # entries: union over cores of (t-window, b) touches; force every
    # (g, b, t) to have >= 1 entry (sacc init at t=0, finalize at t=NQ-1)
    touched = [np.zeros((int(nwt[t]), GRP), dtype=bool) for t in range(NQ)]
    for c in range(NC):
        for t in range(NQ):
            sb = per_core[c]["blk"][t].reshape(-1, 128)
            for b in range(GRP):
                touched[t][:, b] |= (sb == b).any(axis=1)
    for g in range(NGRP):
        for t in range(NQ):
            w0, n = int(wbase[g, t]), int(nw2[g, t])
            for b in range(GRP):
                if not touched[t][w0:w0 + n, b].any():
                    touched[t][w0, b] = True

    # entry ids in consumption order (t, g, w, b) + start/stop per (g,b,t)
    sched = []   # sched[t][g] = list of (wg, [(ent, b, st, sp)])
    nent = 0
    for t in range(NQ):
        st_g = []
        for g in range(NGRP):
            cellw = []
            went = {}
            w0, n = int(wbase[g, t]), int(nw2[g, t])
            for lw in range(n):
                ents = []
                for b in range(GRP):
                    if touched[t][w0 + lw, b]:
                        ents.append([nent, b, False, False])
                        went.setdefault(b, []).append((len(cellw), len(ents) - 1))
                        nent += 1
                cellw.append((w0 + lw, ents))
            for b, lst in went.items():
                wi, ei = lst[0]
                cellw[wi][1][ei][2] = True
                wi, ei = lst[-1]
                cellw[wi][1][ei][3] = True
            st_g.append(cellw)
        sched.append(st_g)

    for c in range(NC):
        scol = np.full((nent, 128), -1.0, dtype=np.float32)
        for t in range(NQ):
            for g in range(NGRP):
                for (wg, ents) in sched[t][g]:
                    sb = per_core[c]["blk"][t][wg * 128:(wg + 1) * 128]
                    ss_ = per_core[c]["slot"][t][wg * 128:(wg + 1) * 128]
                    for (ent, b, _, _) in ents:
                        scol[ent] = np.where(sb == b, ss_, -1).astype(np.float32)
        per_core[c]["slotcol"] = scol

    return {"nw2": nw2, "wbase": wbase, "nwt": nwt, "sched": sched,
            "nent": nent, "per_core": per_core}


def host_prep(cfg, src, dst):
    src = np.asarray(src).astype(np.int64)
    dst = np.asarray(dst).astype(np.int64)
    l1 = _prep_l1(cfg, src, dst)
    l2 = _prep_l2(cfg, src, dst)
    return l1, l2


def build(cfg, l1, l2):
    N, IN, HID = cfg.N, cfg.IN, cfg.HID
    NW1 = l1["NW1"]
    nwt = [int(x) for x in l2["nwt"]]
    NENT2 = l2["nent"]

    nc = bacc.Bacc("TRN2", target_bir_lowering=False)

    xs_d = nc.dram_tensor("xs", [128, NW1 * IN], FP8, kind="ExternalInput")
    slot1_d = nc.dram_tensor("slot1", [128, NW1], F32, kind="ExternalInput")
    idx_d = [nc.dram_tensor(f"idx{t}", [128, max(nwt[t] * 8, 8)], I16,
                            kind="ExternalInput") for t in range(NQ)]
    slot2_d = nc.dram_tensor("slot2", [128, NENT2], F32, kind="ExternalInput")
    segid_d = nc.dram_tensor("segid", [128, cfg.NBLK], F32, kind="ExternalInput")
    iota_d = nc.dram_tensor("iota128", [128, 128], BF16, kind="ExternalInput")
    iotas_d = nc.dram_tensor("iota_seg", [128, cfg.NSEGCH * 128], FP16, kind="ExternalInput")
    ident_d = nc.dram_tensor("ident", [128, 128], BF16, kind="ExternalInput")
    identf_d = nc.dram_tensor("identf", [128, 128], FP8, kind="ExternalInput")
    ident32_d = nc.dram_tensor("ident32", [128, 128], F32, kind="ExternalInput")
    w1_d = nc.dram_tensor("W1s", [IN, HID], BF16, kind="ExternalInput")
    w2_d = nc.dram_tensor("W2s", [HID, HID], BF16, kind="ExternalInput")
    b1_d = nc.dram_tensor("b1col", [128, 1], F32, kind="ExternalInput")
    b2_d = nc.dram_tensor("b2col", [128, 1], F32, kind="ExternalInput")
    ones_d = nc.dram_tensor("onescol", [128, 1], BF16, kind="ExternalInput")

    s2_sh = [nc.dram_tensor(f"s2_sh{q}", [cfg.QROWS, HID], FP8)
             for q in range(NQ)]
    s2_full = [nc.dram_tensor(f"s2_full{q}", [cfg.SUBROWS, HID], FP8,
                              addr_space="Shared") for q in range(NQ)]
    s2_pad = [nc.dram_tensor(f"s2_pad{q}", [cfg.SUBROWS, 2 * HID], FP8)
              for q in range(NQ)]
    out_d = nc.dram_tensor("pooled", [cfg.NSEGCH * 128, HID + 1], F32,
                           kind="ExternalOutput")

    KIN = IN // 128
    nw1 = [int(x) for x in l1["nw1"]]
    base1 = [int(x) for x in l1["base1"]]

    with tile.TileContext(nc) as tc, ExitStack() as ctx:
        const = ctx.enter_context(tc.tile_pool(name="const", bufs=1))
        xsp = ctx.enter_context(tc.tile_pool(name="xsp", bufs=3))
        sp = ctx.enter_context(tc.tile_pool(name="sp", bufs=20))
        hp = ctx.enter_context(tc.tile_pool(name="hp", bufs=3))
        saccp = ctx.enter_context(tc.tile_pool(name="saccp", bufs=1))
        normp = ctx.enter_context(tc.tile_pool(name="normp", bufs=1))

        # ---- constants ----
        iota128 = const.tile([128, 128], BF16)
        nc.sync.dma_start(iota128[:], iota_d[:])
        iotaseg = const.tile([128, cfg.NSEGCH * 128], FP16)
        nc.sync.dma_start(iotaseg[:], iotas_d[:])
        ident = const.tile([128, 128], BF16)
        nc.sync.dma_start(ident[:], ident_d[:])
        identf = const.tile([128, 128], FP8)
        nc.sync.dma_start(identf[:], identf_d[:])
        ident32 = const.tile([128, 128], F32)
        nc.sync.dma_start(ident32[:], ident32_d[:])
        segid = const.tile([128, cfg.NBLK], F32)
        nc.sync.dma_start(segid[:], segid_d[:])
        w1_sb = [const.tile([128, HID], BF16, tag=f"w1_{k}", name=f"w1_{k}")
                 for k in range(KIN)]
        for k in range(KIN):
            nc.sync.dma_start(w1_sb[k][:], w1_d[k * 128:(k + 1) * 128, :])
        w2_sb = const.tile([128, HID], BF16)
        nc.sync.dma_start(w2_sb[:], w2_d[:])
        b1c = const.tile([128, 1], F32)
        nc.sync.dma_start(b1c[:], b1_d[:])
        b2c = const.tile([128, 1], F32)
        nc.sync.dma_start(b2c[:], b2_d[:])
        onesc = const.tile([128, 1], BF16)
        nc.sync.dma_start(onesc[:], ones_d[:])
        slot1 = const.tile([128, NW1], F32)
        nc.sync.dma_start(slot1[:], slot1_d[:])
        slot2 = const.tile([128, NENT2], F32)
        nc.sync.dma_start(slot2[:], slot2_d[:])

        # ================= layer 1 =================
        ctx_l1 = ctx.enter_context(ExitStack())
        ps_acc = ctx_l1.enter_context(tc.tile_pool(name="ps_acc", bufs=3, space="PSUM"))
        ps_h = ctx_l1.enter_context(tc.tile_pool(name="ps_h", bufs=1, space="PSUM"))
        ps_s2 = ctx_l1.enter_context(tc.tile_pool(name="ps_s2", bufs=1, space="PSUM"))

        s2n_cur = [None]

        def l1_block(nb, acc):
            # acc: PSUM [128, 256] f32 FEATURE-major (aggT chunk k at cols
            # k*128..): no PE transposes anywhere in L1 (transposes are
            # serialized against in-flight collectives by the tile layer).
            h_ps = ps_h.tile([128, 128], F32, tag="hps", name="h_ps")
            for k in range(KIN):
                xt = hp.tile([128, 128], BF16, tag="xt", name="xt")
                nc.scalar.activation(xt[:], acc[:, k * 128:(k + 1) * 128], AF.Copy)
                nc.tensor.matmul(h_ps[:], w1_sb[k][:], xt[:],
                                 start=(k == 0), stop=(k == KIN - 1))
            h1t = hp.tile([128, 128], BF16, tag="h1t", name="h1t")
            nc.scalar.activation(h1t[:], h_ps[:], AF.Tanh, bias=b1c[:, 0:1])
            # node-major s2 directly: out[slot, hid] = h1T^T @ W2
            s2_ps = ps_s2.tile([128, 128], F32, tag="s2ps", name="s2_ps")
            nc.tensor.matmul(s2_ps[:], h1t[:], w2_sb[:], start=True, stop=True)
            jb = nb % SB
            if jb == 0:
                s2n_cur[0] = hp.tile([128, SB * 128], FP8, tag="s2n", name="s2n")
            s2n = s2n_cur[0]
            nc.scalar.activation(s2n[:, jb * 128:(jb + 1) * 128], s2_ps[:], AF.Copy)
            if jb == SB - 1:
                q = nb // cfg.QBLK
                r0 = ((nb - SB + 1) % cfg.QBLK) * 128
                nc.scalar.dma_start(
                    s2_sh[q][r0:r0 + SB * 128, :].rearrange(
                        "(j p) f -> p j f", j=SB),
                    s2n[:].rearrange("p (j f) -> p j f", f=128))

        def expand(q):
            ER = 8192   # expand rows per DMA (SWDGE <16384-desc limit)
            for e0 in range(0, cfg.SUBROWS, ER):
                nc.gpsimd.dma_start(s2_pad[q][e0:e0 + ER, 0:HID],
                                    s2_full[q][e0:e0 + ER, :])

        win_blk = []
        for b in range(cfg.NBLK):
            win_blk += [b] * nw1[b]
        acc_cur = {}
        for w0 in range(0, NW1, WB1):
            nwb = min(WB1, NW1 - w0)
            eb = xsp.tile([128, WB1 * IN], FP8, tag="xs", name="xs")
            nc.sync.dma_start(eb[:, :nwb * IN],
                              xs_d[:, w0 * IN:(w0 + nwb) * IN])
            m1s = []
            for j in range(nwb):
                w = w0 + j
                m1 = sp.tile([128, 128], BF16, tag="m1", name="m1")
                nc.vector.tensor_scalar(m1[:], iota128[:], slot1[:, w:w + 1],
                                        None, ALU.is_equal)
                m1s.append(m1)
            for j in range(nwb):
                w = w0 + j
                b = win_blk[w]
                if b not in acc_cur:
                    acc_cur[b] = ps_acc.tile([128, KIN * 128], F32, tag="acc",
                                             name="acc")
                acc = acc_cur[b]
                for k in range(KIN):
                    nc.tensor.matmul(
                        acc[:, k * 128:(k + 1) * 128],
                        eb[:, j * IN + k * 128:j * IN + (k + 1) * 128],
                        m1s[j][:],
                        start=(w == base1[b]),
                        stop=(w == base1[b] + nw1[b] - 1))
                if w == base1[b] + nw1[b] - 1:
                    l1_block(b, acc)
                    del acc_cur[b]
                    if (b + 1) % cfg.QBLK == 0:
                        q = b // cfg.QBLK
                        nc.gpsimd.collective_compute(
                            "AllGather", ALU.bypass,
                            ins=[s2_sh[q].ap().opt()],
                            outs=[s2_full[q].ap().opt()],
                            replica_groups=[list(range(cfg.NC))])
                        # expand table 0 after AG1 is issued (exp0's wait
                        # on AG0 is short by then and nothing queues behind
                        # it except later AGs, whose starts are data-gated)
                        if q == 1:
                            expand(0)

        # ================= layer 2 =================
        ctx_l1.close()
        ctx_l2 = ctx.enter_context(ExitStack())
        pacc_p = ctx_l2.enter_context(tc.tile_pool(name="pacc", bufs=2, space="PSUM"))
        ps_pool = ctx_l2.enter_context(tc.tile_pool(name="ps_pool", bufs=1, space="PSUM"))
        ps_ss = ctx_l2.enter_context(tc.tile_pool(name="ps_ss", bufs=1, space="PSUM"))
        ps_h2 = ctx_l2.enter_context(tc.tile_pool(name="ps_h2", bufs=2, space="PSUM"))
        ebp = ctx_l2.enter_context(tc.tile_pool(name="ebp", bufs=6))
        idxp = ctx_l2.enter_context(tc.tile_pool(name="idxp", bufs=3))
        htp = ctx_l2.enter_context(tc.tile_pool(name="htp", bufs=4))

        sacc = saccp.tile([128, cfg.NBLK * 128], F32)
        h2_all = saccp.tile([128, cfg.NBLK * 128], BF16, name="h2_all")
        ss_all = ps_ss.tile([128, cfg.NBLK], F32, name="ss_all")
        scale = normp.tile([128, cfg.NBLK], F32)
        pool_all = ps_pool.tile([128, cfg.NSEGCH * (HID + 1)], F32, name="pool_all")
        pool_ps = [pool_all[:, s * (HID + 1):(s + 1) * (HID + 1)]
                   for s in range(cfg.NSEGCH)]

        cur_tile = [None] * NQ
        cur_w0 = [0] * NQ
        cur_it = [None] * NQ
        cur_it_w0 = [-1] * NQ

        def ensure_window(t, wg):
            if cur_tile[t] is None or wg >= cur_w0[t] + WB:
                w0 = (wg // WB) * WB
                iw0 = (w0 // (WB * IB)) * (WB * IB)
                if cur_it_w0[t] != iw0:
                    niw = min(WB * IB, nwt[t] - iw0)
                    it = idxp.tile([128, WB * IB * 8], I16, tag="it", name="it")
                    nc.sync.dma_start(it[:, :niw * 8],
                                      idx_d[t][:, iw0 * 8:(iw0 + niw) * 8])
                    cur_it[t] = it
                    cur_it_w0[t] = iw0
                nwin = min(WB, nwt[t] - w0)
                jo = (w0 - iw0) * 8
                eb = ebp.tile([128, WB * 2 * HID], FP8, tag="eb", name="eb")
                nc.gpsimd.dma_gather(
                    out_ap=eb[:, :nwin * 2 * HID].rearrange(
                        "p (n f) -> p n f", f=2 * HID),
                    in_ap=s2_pad[t][:, :],
                    idxs_ap=cur_it[t][:, jo:jo + nwin * 8],
                    num_idxs=nwin * 128,
                    num_idxs_reg=nwin * 128,
                    elem_size=2 * HID)
                cur_tile[t] = eb
                cur_w0[t] = w0
            return cur_tile[t], wg - cur_w0[t]

        FLUSH_AT = (32, 64, 96, 128)
        h2_pend = []

        def flush_logmap():
            if not h2_pend:
                return
            b0 = h2_pend[0]
            nbk = len(h2_pend)
            ss = ss_all[:, b0:b0 + nbk]
            na = normp.tile([128, 32], F32, tag="na", name="na")
            nc.vector.tensor_scalar_max(na[:, :nbk], ss, MIN_SS)
            nrm = normp.tile([128, 32], F32, tag="nrm", name="nrm")
            nc.scalar.activation(nrm[:, :nbk], na[:, :nbk], AF.Sqrt)
            ncl = normp.tile([128, 32], F32, tag="ncl", name="ncl")
            nc.vector.tensor_scalar_min(ncl[:, :nbk], nrm[:, :nbk], MAXNORM)
            om = normp.tile([128, 32], F32, tag="om", name="om")
            nc.vector.tensor_scalar(om[:, :nbk], ncl[:, :nbk], -1.0, 1.0,
                                    ALU.mult, ALU.add)
            op_ = normp.tile([128, 32], F32, tag="op", name="op_")
            nc.vector.tensor_scalar_add(op_[:, :nbk], ncl[:, :nbk], 1.0)
            rc = normp.tile([128, 32], F32, tag="rc", name="rc")
            nc.vector.reciprocal(rc[:, :nbk], om[:, :nbk])
            rat = normp.tile([128, 32], F32, tag="rat", name="rat")
            nc.vector.tensor_mul(rat[:, :nbk], op_[:, :nbk], rc[:, :nbk])
            lg = normp.tile([128, 32], F32, tag="lg", name="lg")
            nc.scalar.activation(lg[:, :nbk], rat[:, :nbk], AF.Ln)
            rcn = normp.tile([128, 32], F32, tag="rcn", name="rcn")
            nc.vector.reciprocal(rcn[:, :nbk], nrm[:, :nbk])
            nc.vector.tensor_mul(rcn[:, :nbk], rcn[:, :nbk], lg[:, :nbk])
            nc.vector.tensor_scalar_mul(scale[:, b0:b0 + nbk], rcn[:, :nbk], 0.5)
            for nb in h2_pend:
                ht = htp.tile([128, HID + 1], BF16, tag="ht", name="ht")
                nc.vector.tensor_scalar(ht[:, :HID],
                                        h2_all[:, nb * 128:(nb + 1) * 128],
                                        scale[:, nb:nb + 1], None, ALU.mult)
                nc.vector.memset(ht[:, HID:HID + 1], 1.0)
                sg = sp.tile([128, cfg.NSEGCH * 128], FP16, tag="sg", name="sg")
                nc.vector.tensor_scalar(sg[:], iotaseg[:], segid[:, nb:nb + 1],
                                        None, ALU.is_equal)
                for s in range(cfg.NSEGCH):
                    nc.tensor.matmul(pool_ps[s], sg[:, s * 128:(s + 1) * 128],
                                     ht[:], start=(nb == 0),
                                     stop=(nb == cfg.NBLK - 1))
            h2_pend.clear()

        def l2_flush(t, g, pacc):
            for b in range(GRP):
                nb = g * GRP + b
                pb = pacc[:, b * 128:(b + 1) * 128]
                sl = sacc[:, nb * 128:(nb + 1) * 128]
                if t < NQ - 1:
                    nc.scalar.activation(sl, pb, AF.Copy)
                else:
                    h2t = hp.tile([128, 128], BF16, tag="h2t", name="h2t")
                    nc.scalar.activation(h2t[:], pb, AF.Tanh,
                                         bias=b2c[:, 0:1],
                                         scale=1.0 / S2SCALE)
                    sq = hp.tile([128, 128], BF16, tag="sq", name="sq")
                    nc.scalar.activation(sq[:], h2t[:], AF.Square)
                    nc.tensor.matmul(ss_all[:, nb:nb + 1], sq[:], onesc[:],
                                     start=True, stop=True)
                    h2tr = ps_h2.tile([128, 128], BF16, tag="h2b",
                                      name="h2tr")
                    nc.tensor.transpose(h2tr[:], h2t[:], ident[:])
                    nc.scalar.activation(
                        h2_all[:, nb * 128:(nb + 1) * 128], h2tr[:], AF.Copy)
                    h2_pend.append(nb)
                    if nb + 1 in FLUSH_AT:
                        flush_logmap()

        for t in range(NQ):
            # expand table t just before stage t's gathers (Pool order:
            # [g_{t-1} .., exp_t, g_t ..]); by then AG_t is done or nearly
            # so, and nothing earlier queues behind the wait.
            if t >= 1:
                expand(t)
            # schedule as a flat list of (g, wg, ents); batch masks per
            # gather call (WB windows) ahead of the matmuls
            flat = []
            for g in range(cfg.NGRP):
                for (wg, ents) in l2["sched"][t][g]:
                    flat.append((g, wg, ents))
            gleft = {g: sum(len(e) for (_, e) in l2["sched"][t][g])
                     for g in range(cfg.NGRP)}
            paccs = {}
            for c0 in range(0, len(flat), WB):
                chunk = flat[c0:c0 + WB]
                eb, _ = ensure_window(t, chunk[0][1])
                masks = {}
                for (g, wg, ents) in chunk:
                    for (ent, b, st_f, sp_f) in ents:
                        m = sp.tile([128, 128], BF16, tag="m2", name="m2")
                        nc.vector.tensor_scalar(m[:], iota128[:],
                                                slot2[:, ent:ent + 1],
                                                None, ALU.is_equal)
                        masks[ent] = m
                for (g, wg, ents) in chunk:
                    eb, joff = ensure_window(t, wg)
                    if g not in paccs:
                        paccs[g] = pacc_p.tile([128, GRP * 128], F32,
                                               tag="pacc", name="pacc")
                    for (ent, b, st_f, sp_f) in ents:
                        if st_f and t > 0:
                            # seed the PSUM acc with the running partial
                            nc.tensor.matmul(
                                paccs[g][:, b * 128:(b + 1) * 128],
                                ident32[:],
                                sacc[:, (g * GRP + b) * 128:
                                     (g * GRP + b + 1) * 128],
                                start=True, stop=False)
                            st_f = False
                        nc.tensor.matmul(
                            paccs[g][:, b * 128:(b + 1) * 128],
                            eb[:, joff * 2 * HID:joff * 2 * HID + HID],
                            masks[ent][:], start=st_f, stop=sp_f)
                        gleft[g] -= 1
                        if gleft[g] == 0:
                            l2_flush(t, g, paccs.pop(g))
        flush_logmap()

        for s in range(cfg.NSEGCH):
            po = htp.tile([128, HID + 1], F32, tag="po", name="po")
            nc.vector.tensor_copy(po[:], pool_ps[s])
            nc.sync.dma_start(out_d[s * 128:(s + 1) * 128, :], po[:])

    nc.compile()
    return nc


def host_inputs(cfg, x, seg_ids, W1, b1, W2, b2, l1, l2):
    N, IN, HID = cfg.N, cfg.IN, cfg.HID
    NW1 = l1["NW1"]
    xb = np.ascontiguousarray(
        (np.asarray(x, np.float32) * XSCALE).astype(NP_FP8))
    iota128 = np.tile(np.arange(128, dtype=np.float32), (128, 1)).astype(NP_BF16)
    iotaseg = np.tile(np.arange(cfg.NSEGCH * 128, dtype=np.float32),
                      (128, 1)).astype(np.float16)
    ident = np.eye(128, dtype=np.float32).astype(NP_BF16)
    identf = np.eye(128, dtype=np.float32).astype(NP_FP8)
    w1 = np.ascontiguousarray(
        (np.asarray(W1, np.float32) / XSCALE).astype(NP_BF16))
    w2 = np.ascontiguousarray(
        (np.asarray(W2, np.float32) * S2SCALE).astype(NP_BF16))
    b1c = np.asarray(b1, np.float32).reshape(128, 1)
    b2c = np.asarray(b2, np.float32).reshape(128, 1)
    ones = np.ones((128, 1), np.float32).astype(NP_BF16)
    seg = np.asarray(seg_ids, np.float32)

    maps = []
    for c in range(cfg.NC):
        pc1 = l1["per_core"][c]
        sidx = pc1["srcidx"]
        rows = xb[np.maximum(sidx, 0)]                     # [NW1*128, 256]
        rows[sidx < 0] = 0
        xs = np.ascontiguousarray(
            rows.reshape(NW1, 128, IN).transpose(1, 0, 2).reshape(128, NW1 * IN))
        slot1 = np.ascontiguousarray(
            pc1["slot"].reshape(NW1, 128).T.astype(np.float32))

        pc2 = l2["per_core"][c]
        idxs = {}
        for t in range(NQ):
            ids = pc2["idx"][t]
            iw = ids.astype(np.int16).reshape(-1, 16).T
            iw = np.tile(iw, (8, 1)).copy()
            idxs[f"idx{t}"] = np.ascontiguousarray(iw.astype(np.int16))

        segc = seg[c * cfg.SHARD:(c + 1) * cfg.SHARD].reshape(cfg.NBLK, 128).T
        maps.append({
            "xs": xs,
            "slot1": slot1,
            **idxs,
            "slot2": np.ascontiguousarray(pc2["slotcol"].T),
            "segid": np.ascontiguousarray(segc.astype(np.float32)),
            "iota128": iota128,
            "iota_seg": iotaseg,
            "ident": ident,
            "identf": identf,
            "ident32": np.eye(128, dtype=np.float32),
            "W1s": w1,
            "W2s": w2,
            "b1col": b1c,
            "b2col": b2c,
            "onescol": ones,
        })
    return maps


def host_epilogue(cfg, partials, batch_size, max_comments):
    acc = np.zeros_like(partials[0], dtype=np.float64)
    for p in partials:
        acc += p.astype(np.float64)
    acc = acc.astype(np.float32)
    nseg = cfg.NSEG
    sums = acc[:nseg, :cfg.HID]
    counts = acc[:nseg, cfg.HID]
    agg = sums / np.maximum(counts, 1.0)[:, None]
    ss = np.maximum(np.sum(agg * agg, axis=1), MIN_SS).astype(np.float32)
    norm = np.sqrt(ss)
    y = agg * (np.tanh(norm) / norm)[:, None]
    ssy = np.maximum(np.sum(y * y, axis=1), MIN_SS).astype(np.float32)
    ny = np.sqrt(ssy)
    f = np.where(ny > MAXNORM, MAXNORM / ny, 1.0).astype(np.float32)
    y = y * f[:, None]
    return y.reshape(int(batch_size), int(max_comments), cfg.HID)


# ====================================================================
# Harness entry point
# ====================================================================

_CACHE = {}


def kernel(x, src, dst, seg_ids, W1, b1, W2, b2, batch_size, max_comments):
    """Full-input GNN ComEnc kernel on 8 Trainium2 NeuronCores."""
    from concourse.bass_utils import run_bass_kernel_spmd

    x = np.asarray(x, dtype=np.float32)
    src = np.asarray(src).astype(np.int64)
    dst = np.asarray(dst).astype(np.int64)
    seg_ids = np.asarray(seg_ids).astype(np.int64)
    W1 = np.asarray(W1, dtype=np.float32)
    b1 = np.asarray(b1, dtype=np.float32)
    W2 = np.asarray(W2, dtype=np.float32)
    b2 = np.asarray(b2, dtype=np.float32)
    bs = int(np.asarray(batch_size))
    mc = int(np.asarray(max_comments))

    n_nodes, in_dim = x.shape
    hid = W1.shape[1]
    nseg = bs * mc
    n_cores = 8

    cfg = Cfg(n_nodes, in_dim, hid, nseg, n_cores)
    l1, l2 = host_prep(cfg, src, dst)

    key = (n_nodes, in_dim, hid, nseg, l1["NW1"], l2["nent"],
           tuple(int(v) for v in l2["nwt"]))
    if key in _CACHE:
        nc = _CACHE[key]
    else:
        nc = build(cfg, l1, l2)
        _CACHE.clear()
        _CACHE[key] = nc

    maps = host_inputs(cfg, x, seg_ids, W1, b1, W2, b2, l1, l2)
    res = run_bass_kernel_spmd(nc, maps, core_ids=list(range(n_cores)))
    partials = [r["pooled"] for r in res.results]
    out = host_epilogue(cfg, partials, bs, mc)
    return np.ascontiguousarray(out.astype(np.float32))


# revision 21
# speedup vs baseline: 1.1491x; 1.1288x over previous
"""GNN message-passing kernel for Trainium2 (8 NeuronCores, SPMD) — v3.

Computation (see reference):
  h1 = tanh(A x @ W1 + b1)          [A(xW) = (Ax)W]
  s2 = h1 @ W2
  h2 = tanh(A s2 + b2)
  ht = logmap0(proj(h2))            (rowwise scale)
  pooled = segment mean over seg_ids, then expmap0/proj (host epilogue)

Sharding: nodes split contiguously over cores (dst-shard), SHARD=16384.

v3 structure per core:
  L1  streams host-pregathered x rows (bf16) in per-block windows (one
      128-edge window per dst block, pads masked).  One DVE is_equal
      one-hot per window feeds one PE matmul into the node-major block
      acc [slot, 256].  Epilogue transposes the acc, h1T = W1^T accT
      (Act tanh with per-partition bias), s2T = (4*W2)^T h1T, transpose
      to node-major fp8e3 s2 rows (scaled x4 for fp8 range).
  Exchange: FOUR fp8 AllGathers (one per quarter of the node space,
      issued as soon as that quarter's L1 blocks finish), each expanded
      by a DRAM->DRAM DMA into a 256B-strided padded table so dma_gather
      can fetch 256-byte descriptors legally.
  L2  four chunks (= exchange stages = int16 sub-tables).  Contiguous
      per-table gather streams (8x128 rows per call, batched idx loads),
      DVE slot masks, feature-major accumulation (lhsT = gathered fp8
      rows), PSUM acc per (group,block) within a chunk, SBUF partial
      across chunks; finalize = add + Act tanh(scale=0.25, bias) ->
      norms via PE ones-matmul -> batched logmap scale -> transpose ->
      pooling via fp16 seg masks.
"""

import numpy as np
import ml_dtypes
from contextlib import ExitStack

import concourse.bass as bass
from concourse.instruction_name_ordered_set import InstructionNameOrderedSet
import concourse.tile as tile
import concourse.bacc as bacc
from concourse import mybir

BF16 = mybir.dt.bfloat16
FP16 = mybir.dt.float16
FP8 = mybir.dt.float8e3
F32 = mybir.dt.float32
I16 = mybir.dt.int16
AF = mybir.ActivationFunctionType
ALU = mybir.AluOpType

NP_FP8 = ml_dtypes.float8_e3m4
NP_BF16 = ml_dtypes.bfloat16

MAXNORM = 1.0 - 1e-5
MIN_SS = 1e-15
XSCALE = 8.0     # x shipped as x*XSCALE in fp8, W1 shipped as W1/XSCALE
S2SCALE = 4.0    # s2 stored as s2*S2SCALE in fp8 (undone in the L2 tanh)

GRP = 8          # dst blocks per L2 PSUM group
WB = 8           # windows per L2 gather call (1024-idx hw ring limit)
WB1 = 32         # windows per L1 stream DMA
IB = 8           # gather calls per idx DMA
SB = 4           # s2 blocks per spill DMA
NQ = 4           # exchange stages / L2 chunks / sub-tables


class Cfg:
    def __init__(self, n_nodes, in_dim, hid, n_seg, n_cores):
        self.N = n_nodes
        self.IN = in_dim
        self.HID = hid
        self.NSEG = n_seg
        self.NC = n_cores
        self.SHARD = n_nodes // n_cores
        self.NBLK = self.SHARD // 128
        self.NGRP = self.NBLK // GRP
        self.NSEGCH = (n_seg + 127) // 128
        self.QROWS = self.SHARD // NQ            # rows per core per stage
        self.SUBROWS = self.NC * self.QROWS      # rows per sub-table (32768)
        self.QBLK = self.NBLK // NQ              # L1 blocks per stage


def _prep_l1(cfg, src, dst):
    """Per-block windows (1 block per window). Streams: srcidx / slot
    (pad: srcidx=-1, slot=-1)."""
    NC, NBLK = cfg.NC, cfg.NBLK
    core = dst // cfg.SHARD
    nb = (dst % cfg.SHARD) // 128
    slot = dst % 128
    cnt = np.zeros((NC, NBLK), dtype=np.int64)
    np.add.at(cnt, (core, nb), 1)
    nw1 = (cnt.max(axis=0) + 127) // 128
    nw1 = np.maximum(nw1, 1)
    base1 = np.concatenate([[0], np.cumsum(nw1)[:-1]])
    NW1 = int(nw1.sum())
    TOT = NW1 * 128
    order = np.lexsort((slot, nb, core))
    per_core = []
    for c in range(NC):
        sel = order[core[order] == c]
        sidx = np.full(TOT, -1, dtype=np.int64)
        sslot = np.full(TOT, -1, dtype=np.int64)
        ep = 0
        for b in range(NBLK):
            n = int(cnt[c, b])
            pos = int(base1[b]) * 128
            if n:
                s = sel[ep:ep + n]
                sidx[pos:pos + n] = src[s]
                sslot[pos:pos + n] = slot[s]
                ep += n
        assert ep == len(sel)
        per_core.append({"srcidx": sidx, "slot": sslot})
    return {"nw1": nw1, "base1": base1, "NW1": NW1, "per_core": per_core}


def _prep_l2(cfg, src, dst):
    """Cells (g, t): t = quarter of the src shard. Windows per cell padded
    to max over cores; entries per (window, b); per-t contiguous streams."""
    NC, NGRP = cfg.NC, cfg.NGRP
    core = dst // cfg.SHARD
    blk = (dst % cfg.SHARD) // 128
    slot = dst % 128
    g_all = blk // GRP
    b_all = blk % GRP
    t_all = (src % cfg.SHARD) // cfg.QROWS
    pos_sub = (src // cfg.SHARD) * cfg.QROWS + (src % cfg.QROWS)

    cnt = np.zeros((NC, NGRP, NQ), dtype=np.int64)
    np.add.at(cnt, (core, g_all, t_all), 1)
    nw2 = (cnt.max(axis=0) + 127) // 128      # [NGRP, NQ]
    nw2 = np.maximum(nw2, 1)                  # every (g, t) cell exists

    # per-t stream window bases, consumption order (t, g)
    wbase = np.zeros((NGRP, NQ), dtype=np.int64)
    nwt = np.zeros(NQ, dtype=np.int64)
    for t in range(NQ):
        for g in range(NGRP):
            wbase[g, t] = nwt[t]
            nwt[t] += nw2[g, t]

    order = np.lexsort((b_all, g_all, t_all, core))
    per_core = []
    for c in range(NC):
        sel = order[core[order] == c]
        streams_idx = [np.zeros(int(nwt[t]) * 128, dtype=np.int64) for t in range(NQ)]
        streams_slot = [np.full(int(nwt[t]) * 128, -1, dtype=np.int64) for t in range(NQ)]
        streams_blk = [np.full(int(nwt[t]) * 128, -1, dtype=np.int64) for t in range(NQ)]
        ep = 0
        for t in range(NQ):
            for g in range(NGRP):
                n = int(cnt[c, g, t])
                if n == 0:
                    continue
                s = sel[ep:ep + n]
                pos = int(wbase[g, t]) * 128
                streams_idx[t][pos:pos + n] = pos_sub[s]
                streams_slot[t][pos:pos + n] = slot[s]
                streams_blk[t][pos:pos + n] = b_all[s]
                ep += n
        assert ep == len(sel)
        per_core.append({"idx": streams_idx, "slot": streams_slot,
                         "blk": streams_blk})

    # entries: union over cores of (t-window, b) touches; force every
    # (g, b, t) to have >= 1 entry (sacc init at t=0, finalize at t=NQ-1)
    touched = [np.zeros((int(nwt[t]), GRP), dtype=bool) for t in range(NQ)]
    for c in range(NC):
        for t in range(NQ):
            sb = per_core[c]["blk"][t].reshape(-1, 128)
            for b in range(GRP):
                touched[t][:, b] |= (sb == b).any(axis=1)
    for g in range(NGRP):
        for t in range(NQ):
            w0, n = int(wbase[g, t]), int(nw2[g, t])
            for b in range(GRP):
                if not touched[t][w0:w0 + n, b].any():
                    touched[t][w0, b] = True

    # entry ids in consumption order (t, g, w, b) + start/stop per (g,b,t)
    sched = []   # sched[t][g] = list of (wg, [(ent, b, st, sp)])
    nent = 0
    for t in range(NQ):
        st_g = []
        for g in range(NGRP):
            cellw = []
            went = {}
            w0, n = int(wbase[g, t]), int(nw2[g, t])
            for lw in range(n):
                ents = []
                for b in range(GRP):
                    if touched[t][w0 + lw, b]:
                        ents.append([nent, b, False, False])
                        went.setdefault(b, []).append((len(cellw), len(ents) - 1))
                        nent += 1
                cellw.append((w0 + lw, ents))
            for b, lst in went.items():
                wi, ei = lst[0]
                cellw[wi][1][ei][2] = True
                wi, ei = lst[-1]
                cellw[wi][1][ei][3] = True
            st_g.append(cellw)
        sched.append(st_g)

    for c in range(NC):
        scol = np.full((nent, 128), -1.0, dtype=np.float32)
        for t in range(NQ):
            for g in range(NGRP):
                for (wg, ents) in sched[t][g]:
                    sb = per_core[c]["blk"][t][wg * 128:(wg + 1) * 128]
                    ss_ = per_core[c]["slot"][t][wg * 128:(wg + 1) * 128]
                    for (ent, b, _, _) in ents:
                        scol[ent] = np.where(sb == b, ss_, -1).astype(np.float32)
        per_core[c]["slotcol"] = scol

    return {"nw2": nw2, "wbase": wbase, "nwt": nwt, "sched": sched,
            "nent": nent, "per_core": per_core}


def host_prep(cfg, src, dst):
    src = np.asarray(src).astype(np.int64)
    dst = np.asarray(dst).astype(np.int64)
    l1 = _prep_l1(cfg, src, dst)
    l2 = _prep_l2(cfg, src, dst)
    return l1, l2


def build(cfg, l1, l2):
    N, IN, HID = cfg.N, cfg.IN, cfg.HID
    NW1 = l1["NW1"]
    nwt = [int(x) for x in l2["nwt"]]
    NENT2 = l2["nent"]

    nc = bacc.Bacc("TRN2", target_bir_lowering=False)

    xs_d = nc.dram_tensor("xs", [128, NW1 * IN], FP8, kind="ExternalInput")
    slot1_d = nc.dram_tensor("slot1", [128, NW1], F32, kind="ExternalInput")
    idx_d = [nc.dram_tensor(f"idx{t}", [128, max(nwt[t] * 8, 8)], I16,
                            kind="ExternalInput") for t in range(NQ)]
    slot2_d = nc.dram_tensor("slot2", [128, NENT2], F32, kind="ExternalInput")
    segid_d = nc.dram_tensor("segid", [128, cfg.NBLK], F32, kind="ExternalInput")
    iota_d = nc.dram_tensor("iota128", [128, 128], BF16, kind="ExternalInput")
    iotas_d = nc.dram_tensor("iota_seg", [128, cfg.NSEGCH * 128], FP16, kind="ExternalInput")
    ident_d = nc.dram_tensor("ident", [128, 128], BF16, kind="ExternalInput")
    identf_d = nc.dram_tensor("identf", [128, 128], FP8, kind="ExternalInput")
    ident32_d = nc.dram_tensor("ident32", [128, 128], F32, kind="ExternalInput")
    w1_d = nc.dram_tensor("W1s", [IN, HID], BF16, kind="ExternalInput")
    w2_d = nc.dram_tensor("W2s", [HID, HID], BF16, kind="ExternalInput")
    b1_d = nc.dram_tensor("b1col", [128, 1], F32, kind="ExternalInput")
    b2_d = nc.dram_tensor("b2col", [128, 1], F32, kind="ExternalInput")
    ones_d = nc.dram_tensor("onescol", [128, 1], BF16, kind="ExternalInput")

    s2_sh = [nc.dram_tensor(f"s2_sh{q}", [cfg.QROWS, HID], FP8)
             for q in range(NQ)]
    s2_full = [nc.dram_tensor(f"s2_full{q}", [cfg.SUBROWS, HID], FP8,
                              addr_space="Shared") for q in range(NQ)]
    s2_pad = [nc.dram_tensor(f"s2_pad{q}", [cfg.SUBROWS, 2 * HID], FP8)
              for q in range(NQ)]
    out_d = nc.dram_tensor("pooled", [cfg.NSEGCH * 128, HID + 1], F32,
                           kind="ExternalOutput")

    KIN = IN // 128
    nw1 = [int(x) for x in l1["nw1"]]
    base1 = [int(x) for x in l1["base1"]]

    with tile.TileContext(nc) as tc, ExitStack() as ctx:
        const = ctx.enter_context(tc.tile_pool(name="const", bufs=1))
        xsp = ctx.enter_context(tc.tile_pool(name="xsp", bufs=3))
        sp = ctx.enter_context(tc.tile_pool(name="sp", bufs=20))
        hp = ctx.enter_context(tc.tile_pool(name="hp", bufs=3))
        saccp = ctx.enter_context(tc.tile_pool(name="saccp", bufs=1))
        normp = ctx.enter_context(tc.tile_pool(name="normp", bufs=1))

        # ---- constants ----
        iota128 = const.tile([128, 128], BF16)
        nc.sync.dma_start(iota128[:], iota_d[:])
        iotaseg = const.tile([128, cfg.NSEGCH * 128], FP16)
        nc.sync.dma_start(iotaseg[:], iotas_d[:])
        ident = const.tile([128, 128], BF16)
        nc.sync.dma_start(ident[:], ident_d[:])
        identf = const.tile([128, 128], FP8)
        nc.sync.dma_start(identf[:], identf_d[:])
        ident32 = const.tile([128, 128], F32)
        nc.sync.dma_start(ident32[:], ident32_d[:])
        segid = const.tile([128, cfg.NBLK], F32)
        nc.sync.dma_start(segid[:], segid_d[:])
        w1_sb = [const.tile([128, HID], BF16, tag=f"w1_{k}", name=f"w1_{k}")
                 for k in range(KIN)]
        for k in range(KIN):
            nc.sync.dma_start(w1_sb[k][:], w1_d[k * 128:(k + 1) * 128, :])
        w2_sb = const.tile([128, HID], BF16)
        nc.sync.dma_start(w2_sb[:], w2_d[:])
        b1c = const.tile([128, 1], F32)
        nc.sync.dma_start(b1c[:], b1_d[:])
        b2c = const.tile([128, 1], F32)
        nc.sync.dma_start(b2c[:], b2_d[:])
        onesc = const.tile([128, 1], BF16)
        nc.sync.dma_start(onesc[:], ones_d[:])
        slot1 = const.tile([128, NW1], F32)
        nc.sync.dma_start(slot1[:], slot1_d[:])
        slot2 = const.tile([128, NENT2], F32)
        nc.sync.dma_start(slot2[:], slot2_d[:])

        # ================= layer 1 =================
        ctx_l1 = ctx.enter_context(ExitStack())
        ps_acc = ctx_l1.enter_context(tc.tile_pool(name="ps_acc", bufs=3, space="PSUM"))
        ps_h = ctx_l1.enter_context(tc.tile_pool(name="ps_h", bufs=1, space="PSUM"))
        ps_s2 = ctx_l1.enter_context(tc.tile_pool(name="ps_s2", bufs=1, space="PSUM"))

        s2n_cur = [None]

        def l1_block(nb, acc):
            # acc: PSUM [128, 256] f32 FEATURE-major (aggT chunk k at cols
            # k*128..): no PE transposes anywhere in L1 (transposes are
            # serialized against in-flight collectives by the tile layer).
            h_ps = ps_h.tile([128, 128], F32, tag="hps", name="h_ps")
            for k in range(KIN):
                xt = hp.tile([128, 128], BF16, tag="xt", name="xt")
                nc.scalar.activation(xt[:], acc[:, k * 128:(k + 1) * 128], AF.Copy)
                nc.tensor.matmul(h_ps[:], w1_sb[k][:], xt[:],
                                 start=(k == 0), stop=(k == KIN - 1))
            h1t = hp.tile([128, 128], BF16, tag="h1t", name="h1t")
            nc.scalar.activation(h1t[:], h_ps[:], AF.Tanh, bias=b1c[:, 0:1])
            # node-major s2 directly: out[slot, hid] = h1T^T @ W2
            s2_ps = ps_s2.tile([128, 128], F32, tag="s2ps", name="s2_ps")
            nc.tensor.matmul(s2_ps[:], h1t[:], w2_sb[:], start=True, stop=True)
            jb = nb % SB
            if jb == 0:
                s2n_cur[0] = hp.tile([128, SB * 128], FP8, tag="s2n", name="s2n")
            s2n = s2n_cur[0]
            nc.scalar.activation(s2n[:, jb * 128:(jb + 1) * 128], s2_ps[:], AF.Copy)
            if jb == SB - 1:
                q = nb // cfg.QBLK
                r0 = ((nb - SB + 1) % cfg.QBLK) * 128
                nc.scalar.dma_start(
                    s2_sh[q][r0:r0 + SB * 128, :].rearrange(
                        "(j p) f -> p j f", j=SB),
                    s2n[:].rearrange("p (j f) -> p j f", f=128))

        def expand(q, after=None):
            ER = 8192   # expand rows per DMA (SWDGE <16384-desc limit)
            for e0 in range(0, cfg.SUBROWS, ER):
                ei = nc.gpsimd.dma_start(s2_pad[q][e0:e0 + ER, 0:HID],
                                         s2_full[q][e0:e0 + ER, :])
                if after is not None:
                    # ordering-only dep: keep the scheduler from hoisting
                    # this expand (and its collective wait) ahead of the
                    # previous stage's gathers on the in-order Pool queue
                    ns = InstructionNameOrderedSet()
                    ns.add(after)
                    ei.ins.add_nosync_dependencies_from(ns)

        win_blk = []
        for b in range(cfg.NBLK):
            win_blk += [b] * nw1[b]
        acc_cur = {}
        for w0 in range(0, NW1, WB1):
            nwb = min(WB1, NW1 - w0)
            eb = xsp.tile([128, WB1 * IN], FP8, tag="xs", name="xs")
            nc.sync.dma_start(eb[:, :nwb * IN],
                              xs_d[:, w0 * IN:(w0 + nwb) * IN])
            m1s = []
            for j in range(nwb):
                w = w0 + j
                m1 = sp.tile([128, 128], BF16, tag="m1", name="m1")
                nc.vector.tensor_scalar(m1[:], iota128[:], slot1[:, w:w + 1],
                                        None, ALU.is_equal)
                m1s.append(m1)
            for j in range(nwb):
                w = w0 + j
                b = win_blk[w]
                if b not in acc_cur:
                    acc_cur[b] = ps_acc.tile([128, KIN * 128], F32, tag="acc",
                                             name="acc")
                acc = acc_cur[b]
                for k in range(KIN):
                    nc.tensor.matmul(
                        acc[:, k * 128:(k + 1) * 128],
                        eb[:, j * IN + k * 128:j * IN + (k + 1) * 128],
                        m1s[j][:],
                        start=(w == base1[b]),
                        stop=(w == base1[b] + nw1[b] - 1))
                if w == base1[b] + nw1[b] - 1:
                    l1_block(b, acc)
                    del acc_cur[b]
                    if (b + 1) % cfg.QBLK == 0:
                        q = b // cfg.QBLK
                        nc.gpsimd.collective_compute(
                            "AllGather", ALU.bypass,
                            ins=[s2_sh[q].ap().opt()],
                            outs=[s2_full[q].ap().opt()],
                            replica_groups=[list(range(cfg.NC))])


        # ================= layer 2 =================
        ctx_l1.close()
        ctx_l2 = ctx.enter_context(ExitStack())
        pacc_p = ctx_l2.enter_context(tc.tile_pool(name="pacc", bufs=2, space="PSUM"))
        ps_pool = ctx_l2.enter_context(tc.tile_pool(name="ps_pool", bufs=1, space="PSUM"))
        ps_ss = ctx_l2.enter_context(tc.tile_pool(name="ps_ss", bufs=1, space="PSUM"))
        ps_h2 = ctx_l2.enter_context(tc.tile_pool(name="ps_h2", bufs=2, space="PSUM"))
        ebp = ctx_l2.enter_context(tc.tile_pool(name="ebp", bufs=6))
        idxp = ctx_l2.enter_context(tc.tile_pool(name="idxp", bufs=3))
        htp = ctx_l2.enter_context(tc.tile_pool(name="htp", bufs=4))

        sacc = saccp.tile([128, cfg.NBLK * 128], F32)
        h2_all = saccp.tile([128, cfg.NBLK * 128], BF16, name="h2_all")
        ss_all = ps_ss.tile([128, cfg.NBLK], F32, name="ss_all")
        scale = normp.tile([128, cfg.NBLK], F32)
        pool_all = ps_pool.tile([128, cfg.NSEGCH * (HID + 1)], F32, name="pool_all")
        pool_ps = [pool_all[:, s * (HID + 1):(s + 1) * (HID + 1)]
                   for s in range(cfg.NSEGCH)]

        last_gather = [None]
        cur_tile = [None] * NQ
        cur_w0 = [0] * NQ
        cur_it = [None] * NQ
        cur_it_w0 = [-1] * NQ

        def ensure_window(t, wg):
            if cur_tile[t] is None or wg >= cur_w0[t] + WB:
                w0 = (wg // WB) * WB
                iw0 = (w0 // (WB * IB)) * (WB * IB)
                if cur_it_w0[t] != iw0:
                    niw = min(WB * IB, nwt[t] - iw0)
                    it = idxp.tile([128, WB * IB * 8], I16, tag="it", name="it")
                    nc.sync.dma_start(it[:, :niw * 8],
                                      idx_d[t][:, iw0 * 8:(iw0 + niw) * 8])
                    cur_it[t] = it
                    cur_it_w0[t] = iw0
                nwin = min(WB, nwt[t] - w0)
                jo = (w0 - iw0) * 8
                eb = ebp.tile([128, WB * 2 * HID], FP8, tag="eb", name="eb")
                last_gather[0] = nc.gpsimd.dma_gather(
                    out_ap=eb[:, :nwin * 2 * HID].rearrange(
                        "p (n f) -> p n f", f=2 * HID),
                    in_ap=s2_pad[t][:, :],
                    idxs_ap=cur_it[t][:, jo:jo + nwin * 8],
                    num_idxs=nwin * 128,
                    num_idxs_reg=nwin * 128,
                    elem_size=2 * HID)
                cur_tile[t] = eb
                cur_w0[t] = w0
            return cur_tile[t], wg - cur_w0[t]

        FLUSH_AT = (32, 64, 96, 128)
        h2_pend = []

        def flush_logmap():
            if not h2_pend:
                return
            b0 = h2_pend[0]
            nbk = len(h2_pend)
            ss = ss_all[:, b0:b0 + nbk]
            na = normp.tile([128, 32], F32, tag="na", name="na")
            nc.vector.tensor_scalar_max(na[:, :nbk], ss, MIN_SS)
            nrm = normp.tile([128, 32], F32, tag="nrm", name="nrm")
            nc.scalar.activation(nrm[:, :nbk], na[:, :nbk], AF.Sqrt)
            ncl = normp.tile([128, 32], F32, tag="ncl", name="ncl")
            nc.vector.tensor_scalar_min(ncl[:, :nbk], nrm[:, :nbk], MAXNORM)
            om = normp.tile([128, 32], F32, tag="om", name="om")
            nc.vector.tensor_scalar(om[:, :nbk], ncl[:, :nbk], -1.0, 1.0,
                                    ALU.mult, ALU.add)
            op_ = normp.tile([128, 32], F32, tag="op", name="op_")
            nc.vector.tensor_scalar_add(op_[:, :nbk], ncl[:, :nbk], 1.0)
            rc = normp.tile([128, 32], F32, tag="rc", name="rc")
            nc.vector.reciprocal(rc[:, :nbk], om[:, :nbk])
            rat = normp.tile([128, 32], F32, tag="rat", name="rat")
            nc.vector.tensor_mul(rat[:, :nbk], op_[:, :nbk], rc[:, :nbk])
            lg = normp.tile([128, 32], F32, tag="lg", name="lg")
            nc.scalar.activation(lg[:, :nbk], rat[:, :nbk], AF.Ln)
            rcn = normp.tile([128, 32], F32, tag="rcn", name="rcn")
            nc.vector.reciprocal(rcn[:, :nbk], nrm[:, :nbk])
            nc.vector.tensor_mul(rcn[:, :nbk], rcn[:, :nbk], lg[:, :nbk])
            nc.vector.tensor_scalar_mul(scale[:, b0:b0 + nbk], rcn[:, :nbk], 0.5)
            for nb in h2_pend:
                ht = htp.tile([128, HID + 1], BF16, tag="ht", name="ht")
                nc.vector.tensor_scalar(ht[:, :HID],
                                        h2_all[:, nb * 128:(nb + 1) * 128],
                                        scale[:, nb:nb + 1], None, ALU.mult)
                nc.vector.memset(ht[:, HID:HID + 1], 1.0)
                sg = sp.tile([128, cfg.NSEGCH * 128], FP16, tag="sg", name="sg")
                nc.vector.tensor_scalar(sg[:], iotaseg[:], segid[:, nb:nb + 1],
                                        None, ALU.is_equal)
                for s in range(cfg.NSEGCH):
                    nc.tensor.matmul(pool_ps[s], sg[:, s * 128:(s + 1) * 128],
                                     ht[:], start=(nb == 0),
                                     stop=(nb == cfg.NBLK - 1))
            h2_pend.clear()

        def l2_flush(t, g, pacc):
            for b in range(GRP):
                nb = g * GRP + b
                pb = pacc[:, b * 128:(b + 1) * 128]
                sl = sacc[:, nb * 128:(nb + 1) * 128]
                if t < NQ - 1:
                    nc.scalar.activation(sl, pb, AF.Copy)
                else:
                    h2t = hp.tile([128, 128], BF16, tag="h2t", name="h2t")
                    nc.scalar.activation(h2t[:], pb, AF.Tanh,
                                         bias=b2c[:, 0:1],
                                         scale=1.0 / S2SCALE)
                    sq = hp.tile([128, 128], BF16, tag="sq", name="sq")
                    nc.scalar.activation(sq[:], h2t[:], AF.Square)
                    nc.tensor.matmul(ss_all[:, nb:nb + 1], sq[:], onesc[:],
                                     start=True, stop=True)
                    h2tr = ps_h2.tile([128, 128], BF16, tag="h2b",
                                      name="h2tr")
                    nc.tensor.transpose(h2tr[:], h2t[:], ident[:])
                    nc.scalar.activation(
                        h2_all[:, nb * 128:(nb + 1) * 128], h2tr[:], AF.Copy)
                    h2_pend.append(nb)
                    if nb + 1 in FLUSH_AT:
                        flush_logmap()

        for t in range(NQ):
            # expand table t just before stage t's gathers (Pool order:
            # [g_{t-1} .., exp_t, g_t ..]); pinned behind the previous
            # stage's last gather so its collective wait can't stall them.
            if t == 0:
                expand(0)
            else:
                lg = last_gather[0]
                expand(t, after=lg.ins.name if lg is not None else None)
            # schedule as a flat list of (g, wg, ents); batch masks per
            # gather call (WB windows) ahead of the matmuls
            flat = []
            for g in range(cfg.NGRP):
                for (wg, ents) in l2["sched"][t][g]:
                    flat.append((g, wg, ents))
            gleft = {g: sum(len(e) for (_, e) in l2["sched"][t][g])
                     for g in range(cfg.NGRP)}
            paccs = {}
            for c0 in range(0, len(flat), WB):
                chunk = flat[c0:c0 + WB]
                eb, _ = ensure_window(t, chunk[0][1])
                masks = {}
                for (g, wg, ents) in chunk:
                    for (ent, b, st_f, sp_f) in ents:
                        m = sp.tile([128, 128], BF16, tag="m2", name="m2")
                        nc.vector.tensor_scalar(m[:], iota128[:],
                                                slot2[:, ent:ent + 1],
                                                None, ALU.is_equal)
                        masks[ent] = m
                for (g, wg, ents) in chunk:
                    eb, joff = ensure_window(t, wg)
                    if g not in paccs:
                        paccs[g] = pacc_p.tile([128, GRP * 128], F32,
                                               tag="pacc", name="pacc")
                    for (ent, b, st_f, sp_f) in ents:
                        if st_f and t > 0:
                            # seed the PSUM acc with the running partial
                            nc.tensor.matmul(
                                paccs[g][:, b * 128:(b + 1) * 128],
                                ident32[:],
                                sacc[:, (g * GRP + b) * 128:
                                     (g * GRP + b + 1) * 128],
                                start=True, stop=False)
                            st_f = False
                        nc.tensor.matmul(
                            paccs[g][:, b * 128:(b + 1) * 128],
                            eb[:, joff * 2 * HID:joff * 2 * HID + HID],
                            masks[ent][:], start=st_f, stop=sp_f)
                        gleft[g] -= 1
                        if gleft[g] == 0:
                            l2_flush(t, g, paccs.pop(g))
        flush_logmap()

        for s in range(cfg.NSEGCH):
            po = htp.tile([128, HID + 1], F32, tag="po", name="po")
            nc.vector.tensor_copy(po[:], pool_ps[s])
            nc.sync.dma_start(out_d[s * 128:(s + 1) * 128, :], po[:])

    nc.compile()
    return nc


def host_inputs(cfg, x, seg_ids, W1, b1, W2, b2, l1, l2):
    N, IN, HID = cfg.N, cfg.IN, cfg.HID
    NW1 = l1["NW1"]
    xb = np.ascontiguousarray(
        (np.asarray(x, np.float32) * XSCALE).astype(NP_FP8))
    iota128 = np.tile(np.arange(128, dtype=np.float32), (128, 1)).astype(NP_BF16)
    iotaseg = np.tile(np.arange(cfg.NSEGCH * 128, dtype=np.float32),
                      (128, 1)).astype(np.float16)
    ident = np.eye(128, dtype=np.float32).astype(NP_BF16)
    identf = np.eye(128, dtype=np.float32).astype(NP_FP8)
    w1 = np.ascontiguousarray(
        (np.asarray(W1, np.float32) / XSCALE).astype(NP_BF16))
    w2 = np.ascontiguousarray(
        (np.asarray(W2, np.float32) * S2SCALE).astype(NP_BF16))
    b1c = np.asarray(b1, np.float32).reshape(128, 1)
    b2c = np.asarray(b2, np.float32).reshape(128, 1)
    ones = np.ones((128, 1), np.float32).astype(NP_BF16)
    seg = np.asarray(seg_ids, np.float32)

    maps = []
    for c in range(cfg.NC):
        pc1 = l1["per_core"][c]
        sidx = pc1["srcidx"]
        rows = xb[np.maximum(sidx, 0)]                     # [NW1*128, 256]
        rows[sidx < 0] = 0
        xs = np.ascontiguousarray(
            rows.reshape(NW1, 128, IN).transpose(1, 0, 2).reshape(128, NW1 * IN))
        slot1 = np.ascontiguousarray(
            pc1["slot"].reshape(NW1, 128).T.astype(np.float32))

        pc2 = l2["per_core"][c]
        idxs = {}
        for t in range(NQ):
            ids = pc2["idx"][t]
            iw = ids.astype(np.int16).reshape(-1, 16).T
            iw = np.tile(iw, (8, 1)).copy()
            idxs[f"idx{t}"] = np.ascontiguousarray(iw.astype(np.int16))

        segc = seg[c * cfg.SHARD:(c + 1) * cfg.SHARD].reshape(cfg.NBLK, 128).T
        maps.append({
            "xs": xs,
            "slot1": slot1,
            **idxs,
            "slot2": np.ascontiguousarray(pc2["slotcol"].T),
            "segid": np.ascontiguousarray(segc.astype(np.float32)),
            "iota128": iota128,
            "iota_seg": iotaseg,
            "ident": ident,
            "identf": identf,
            "ident32": np.eye(128, dtype=np.float32),
            "W1s": w1,
            "W2s": w2,
            "b1col": b1c,
            "b2col": b2c,
            "onescol": ones,
        })
    return maps


def host_epilogue(cfg, partials, batch_size, max_comments):
    acc = np.zeros_like(partials[0], dtype=np.float64)
    for p in partials:
        acc += p.astype(np.float64)
    acc = acc.astype(np.float32)
    nseg = cfg.NSEG
    sums = acc[:nseg, :cfg.HID]
    counts = acc[:nseg, cfg.HID]
    agg = sums / np.maximum(counts, 1.0)[:, None]
    ss = np.maximum(np.sum(agg * agg, axis=1), MIN_SS).astype(np.float32)
    norm = np.sqrt(ss)
    y = agg * (np.tanh(norm) / norm)[:, None]
    ssy = np.maximum(np.sum(y * y, axis=1), MIN_SS).astype(np.float32)
    ny = np.sqrt(ssy)
    f = np.where(ny > MAXNORM, MAXNORM / ny, 1.0).astype(np.float32)
    y = y * f[:, None]
    return y.reshape(int(batch_size), int(max_comments), cfg.HID)


# ====================================================================
# Harness entry point
# ====================================================================

_CACHE = {}


def kernel(x, src, dst, seg_ids, W1, b1, W2, b2, batch_size, max_comments):
    """Full-input GNN ComEnc kernel on 8 Trainium2 NeuronCores."""
    from concourse.bass_utils import run_bass_kernel_spmd

    x = np.asarray(x, dtype=np.float32)
    src = np.asarray(src).astype(np.int64)
    dst = np.asarray(dst).astype(np.int64)
    seg_ids = np.asarray(seg_ids).astype(np.int64)
    W1 = np.asarray(W1, dtype=np.float32)
    b1 = np.asarray(b1, dtype=np.float32)
    W2 = np.asarray(W2, dtype=np.float32)
    b2 = np.asarray(b2, dtype=np.float32)
    bs = int(np.asarray(batch_size))
    mc = int(np.asarray(max_comments))

    n_nodes, in_dim = x.shape
    hid = W1.shape[1]
    nseg = bs * mc
    n_cores = 8

    cfg = Cfg(n_nodes, in_dim, hid, nseg, n_cores)
    l1, l2 = host_prep(cfg, src, dst)

    key = (n_nodes, in_dim, hid, nseg, l1["NW1"], l2["nent"],
           tuple(int(v) for v in l2["nwt"]))
    if key in _CACHE:
        nc = _CACHE[key]
    else:
        nc = build(cfg, l1, l2)
        _CACHE.clear()
        _CACHE[key] = nc

    maps = host_inputs(cfg, x, seg_ids, W1, b1, W2, b2, l1, l2)
    res = run_bass_kernel_spmd(nc, maps, core_ids=list(range(n_cores)))
    partials = [r["pooled"] for r in res.results]
    out = host_epilogue(cfg, partials, bs, mc)
    return np.ascontiguousarray(out.astype(np.float32))
